# revision 7
# baseline (speedup 1.0000x reference)
"""Trainium2 Bass kernel for nn_CA_Model (neural cellular automaton).

Data-parallel over batch (8 images -> 8 cores). Per core the [256,256,16]
image lives in SBUF as FOUR fp16 row-shifted copies
  Xc[16*((r+c)%8)+ch, ((r+c)//8)*258 + 1 + w],  c in 0..3
so that every output row pair (rho, rho+4) finds its 3-row conv support on
partition strips {0,1} (group 1) and {2,3} (group 5) of the SAME copy:
the three dj matmuls of the 3x3-perceive + layer-1 fuse run as K=64
tile_position pairs that execute concurrently on disjoint PE row strips.

Layer 2 accumulates eight per-row matmuls (zero-padded W1^T columns) into a
PSUM tile per row-block. Alive maxpools run on a q2-interleaved alpha tile;
vertical max via DMA partition-shifted copies. State updates write the sigma=0
copy; copies 1..3 are refreshed by two batched partition-rotating DMAs per
copy per image half, overlapped with compute of the other half.
"""
import sys
for _p in ("/opt/trn_rl_repo", "/root/.axon_site/_ro/trn_rl_repo"):
    if _p not in sys.path:
        sys.path.append(_p)

import numpy as np

C = 16
HID = 128
H = W = 256
NB = 34            # row blocks in layout (34*8 = 272 slots; rows 0..257+shift used)
NBOUT = 33         # blocks that can hold image rows in sigma-0 layout
FW = 258           # padded row width in free dim
FSZ = NB * FW      # 8772 free elements per partition
NSIG = 4           # shifted copies


def _sobel():
    dx = np.outer([1, 2, 1], [-1, 0, 1]) / 8.0
    f1 = dx.T.astype(np.float32)
    f2 = dx.astype(np.float32)
    return f1, f2


def build_weights(W0, b0, W1):
    import ml_dtypes
    F1, F2 = _sobel()
    W0x, W0y1, W0y2 = W0[:, 0:16], W0[:, 16:32], W0[:, 32:48]
    # A[di][dj]: [HID, C] applied to x[row-1+di, w-1+dj]
    A = [[(np.float32(di == 1 and dj == 1) * W0x
           + F1[di, dj] * W0y1 + F2[di, dj] * W0y2).astype(np.float32)
          for dj in range(3)] for di in range(3)]

    # L1W [128, 3*128]: column block dj holds the K=64 lhsT for the A side
    # (support groups 0,1,2 -> partitions 16*di+c) and, shifted down 64, the
    # B side (groups 4,5,6 -> partitions 64+16*di+c).
    L1W = np.zeros((128, 3 * 128), np.float32)
    for dj in range(3):
        for di in range(3):
            L1W[16 * di:16 * di + 16, 128 * dj:128 * dj + 128] = A[di][dj].T
            L1W[64 + 16 * di:64 + 16 * di + 16, 128 * dj:128 * dj + 128] = A[di][dj].T

    # layer-2 lhsT: for a row with group g, W1pad[g][n, 16g+c] = W1[c, n]
    w1p = []
    for g in range(8):
        Wp = np.zeros((128, 128), np.float32)
        Wp[:, 16 * g:16 * g + 16] = W1.T
        w1p.append(Wp)

    # life-broadcast lhsT variants (q2 layout of LifeF -> X layout), per tb
    life_plan = []
    r_mats = []
    for tb in range(NBOUT):
        plan = []
        buckets = {}
        for g in range(8):
            rho = 8 * tb + g
            if rho < 1 or rho > 256:
                continue
            q = rho - 1
            half, qh = q // 128, q % 128
            buckets.setdefault(half, []).append((g, qh))
        for half, rows in sorted(buckets.items()):
            Rm = np.zeros((128, 128), np.float32)
            for g, qh in rows:
                q2 = (qh % 8) * 16 + qh // 8
                Rm[q2, 16 * g:16 * g + 16] = 1.0
            plan.append((half, len(r_mats)))
            r_mats.append(Rm)
        life_plan.append(plan)

    f16 = np.float16
    return dict(l1w=L1W.astype(f16),
                w1_stack=np.stack(w1p).astype(f16),
                r_stack=np.stack(r_mats).astype(f16),
                life_plan=life_plan,
                b0=b0.reshape(128, 1).astype(np.float32))


def marshal_x(img):
    """[256,256,16] image -> 4 shifted copies [128, FSZ] fp16."""
    out = []
    for c in range(NSIG):
        xp = np.zeros((NB * 8, FW, C), np.float16)
        xp[1 + c:257 + c, 1:257, :] = img
        xc = xp.reshape(NB, 8, FW, C).transpose(1, 3, 0, 2).reshape(128, FSZ)
        out.append(np.ascontiguousarray(xc))
    return out


def unmarshal_x(xc):
    """X0 [128, FSZ] -> [256,256,16] image."""
    xp = xc.reshape(8, C, NB, FW).transpose(2, 0, 3, 1)
    xp = xp.reshape(NB * 8, FW, C)
    return np.ascontiguousarray(xp[1:257, 1:257, :])


def build_program(steps, life_plan, n_r,
                  relu_split=(7, 1, 0), debug_phases=99):
    """relu_split: of every sum(relu_split) relu tiles, how many go to
    (scalar, vector, gpsimd)."""
    import concourse.bass as bass
    import concourse.bacc as bacc
    import concourse.tile as tile
    from concourse import mybir
    F32 = mybir.dt.float32
    F16 = mybir.dt.float16
    AF = mybir.ActivationFunctionType
    ALU = mybir.AluOpType
    nc = bacc.Bacc(None, target_bir_lowering=False, debug=False, num_devices=8,
                   num_swdge_queues=4)

    x_ext = [nc.declare_dram_parameter(f"xc{c}", [128, FSZ], F16, isOutput=False)
             for c in range(NSIG)]
    l1_ext = nc.declare_dram_parameter("l1w", [128, 3 * 128], F16, isOutput=False)
    w1_ext = nc.declare_dram_parameter("w1w", [8, 128, 128], F16, isOutput=False)
    r_ext = nc.declare_dram_parameter("rw", [n_r, 128, 128], F16, isOutput=False)
    b0_ext = nc.declare_dram_parameter("b0w", [128, 1], F32, isOutput=False)
    out_ext = nc.declare_dram_parameter("out", [128, FSZ], F16, isOutput=True)

    with tile.TileContext(nc) as tc:
        with tc.tile_pool(name="hpool", bufs=4) as hpool, \
             tc.tile_pool(name="ph_pool", bufs=2, space="PSUM") as ph_pool, \
             tc.tile_pool(name="pd_pool", bufs=3, space="PSUM") as pd_pool:

            # --- persistent state ---
            X = [nc.alloc_sbuf_tensor(f"X{c}", [128, FSZ], F16).ap()
                 for c in range(NSIG)]
            XN = nc.alloc_sbuf_tensor("XN", [128, FSZ], F16).ap()

            LW1 = nc.alloc_sbuf_tensor("LW1", [128, 3 * 128], F16).ap()
            LW2 = nc.alloc_sbuf_tensor("LW2", [128, 8 * 128], F16).ap()
            LWR = nc.alloc_sbuf_tensor("LWR", [128, n_r * 128], F16).ap()
            w1t = [LW2[:, 128 * g:128 * g + 128] for g in range(8)]
            rt = [LWR[:, 128 * i:128 * i + 128] for i in range(n_r)]
            b0t = nc.alloc_sbuf_tensor("b0t", [128, 1], F32).ap()

            A_pre = nc.alloc_sbuf_tensor("A_pre", [128, 516], F16).ap()
            A_post = nc.alloc_sbuf_tensor("A_post", [128, 516], F16).ap()
            HM = nc.alloc_sbuf_tensor("HM", [128, 512], F16).ap()
            HMu = nc.alloc_sbuf_tensor("HMu", [128, 512], F16).ap()
            HMd = nc.alloc_sbuf_tensor("HMd", [128, 512], F16).ap()
            HMp = nc.alloc_sbuf_tensor("HMp", [128, 512], F16).ap()
            HMpu = nc.alloc_sbuf_tensor("HMpu", [128, 512], F16).ap()
            HMpd = nc.alloc_sbuf_tensor("HMpd", [128, 512], F16).ap()
            # seam scratch: partition 0: [0:258] a128 | [258:516] a129 |
            # [516:774] hm128 | [774:1032] hm129
            SEAM = nc.alloc_sbuf_tensor("SEAM", [128, 1032], F16).ap()
            VMpre = nc.alloc_sbuf_tensor("VMpre", [128, 512], F16).ap()
            VMpost = nc.alloc_sbuf_tensor("VMpost", [128, 512], F16).ap()
            LifeF = nc.alloc_sbuf_tensor("LifeF", [128, 512], F16).ap()
            Zrow = nc.alloc_sbuf_tensor("Zrow", [128, 516], F16).ap()

            # --- loads / init ---
            for c in range(NSIG):
                nc.gpsimd.dma_start(out=X[c][:], in_=x_ext[c][:])
            nc.gpsimd.dma_start(out=LW1[:], in_=l1_ext[:])
            nc.gpsimd.dma_start(out=LW2[:], in_=bass.AP(
                tensor=w1_ext, offset=0,
                ap=[[128, 128], [128 * 128, 8], [1, 128]]))
            nc.gpsimd.dma_start(out=LWR[:], in_=bass.AP(
                tensor=r_ext, offset=0,
                ap=[[128, 128], [128 * 128, n_r], [1, 128]]))
            nc.gpsimd.dma_start(out=b0t[:], in_=b0_ext[:])
            nc.vector.memset(Zrow[:], 0.0)
            nc.vector.memset(SEAM[0:32, :], 0.0)
            nc.vector.memset(A_post[:], 0.0)
            nc.vector.memset(A_pre[:], 0.0)

            relu_ctr = [0]
            rs_total = sum(relu_split)
            rs_s, rs_v = relu_split[0], relu_split[0] + relu_split[1]

            def relu_tile(dst, src):
                k = relu_ctr[0] % rs_total
                relu_ctr[0] += 1
                if k < rs_s:
                    nc.scalar.activation(dst, src, AF.Relu, bias=b0t[:], scale=1.0)
                elif k < rs_v:
                    nc.vector.tensor_scalar(dst, src, b0t[:], 0.0,
                                            op0=ALU.add, op1=ALU.max)
                else:
                    nc.vector.tensor_scalar(dst, src, b0t[:], 0.0,
                                            op0=ALU.add, op1=ALU.max)

            def extract_alpha(dst_A, src_X, halves=(0, 1)):
                # q2-layout: dst_A[gp*16 + j, half*258 + 1 + w] holds alpha of
                # row rho = 128*half + 8j + gp + 1 (alpha: channel 3).
                for half in halves:
                    for gp in range(8):
                        g = (gp + 1) % 8
                        t0 = 16 * half + (1 if gp == 7 else 0)
                        dst = bass.AP(
                            tensor=dst_A.tensor,
                            offset=16 * gp * 516 + 258 * half + 1,
                            ap=[[516, 16], [1, 256]])
                        srcap = bass.AP(
                            tensor=src_X.tensor,
                            offset=(16 * g + 3) * FSZ + t0 * FW + 1,
                            ap=[[FSZ, 1], [FW, 16], [1, 256]])
                        eng = nc.sync if gp % 2 == 0 else nc.gpsimd
                        eng.dma_start(out=dst, in_=srcap)

            def pool_half(dst_VM, src_A, half, hm, hmu, hmd):
                lo, hi = 258 * half, 258 * half + 258
                qlo, qhi = 256 * half, 256 * half + 256
                av = src_A[:, lo:hi]
                nc.vector.tensor_tensor(hm[:, qlo:qhi], av[:, 0:256],
                                        av[:, 2:258], op=ALU.max)
                nc.vector.tensor_tensor(hm[:, qlo:qhi], hm[:, qlo:qhi],
                                        av[:, 1:257], op=ALU.max)
                nc.sync.dma_start(out=hmu[0:112, qlo:qhi], in_=hm[16:128, qlo:qhi])
                nc.sync.dma_start(out=hmu[112:127, qlo:qhi], in_=hm[1:16, qlo:qhi])
                nc.sync.dma_start(out=hmd[16:128, qlo:qhi], in_=hm[0:112, qlo:qhi])
                nc.sync.dma_start(out=hmd[1:16, qlo:qhi], in_=hm[112:127, qlo:qhi])
                if half == 0:
                    nc.sync.dma_start(out=hmu[127:128, 0:256],
                                      in_=SEAM[0:1, 775:1031])
                    nc.sync.dma_start(out=hmd[0:1, 0:256], in_=Zrow[0:1, 0:256])
                else:
                    nc.sync.dma_start(out=hmd[0:1, 256:512],
                                      in_=SEAM[0:1, 517:773])
                    nc.sync.dma_start(out=hmu[127:128, 256:512],
                                      in_=Zrow[0:1, 0:256])
                nc.vector.tensor_tensor(dst_VM[:, qlo:qhi], hm[:, qlo:qhi],
                                        hmu[:, qlo:qhi], op=ALU.max)
                nc.vector.tensor_tensor(dst_VM[:, qlo:qhi], dst_VM[:, qlo:qhi],
                                        hmd[:, qlo:qhi], op=ALU.max)

            def seam_hmax():
                sv = SEAM[0:1, :].rearrange("p (a w) -> p a w", a=4)
                nc.vector.tensor_tensor(sv[:, 2:4, 1:257], sv[:, 0:2, 0:256],
                                        sv[:, 0:2, 2:258], op=ALU.max)
                nc.vector.tensor_tensor(sv[:, 2:4, 1:257], sv[:, 2:4, 1:257],
                                        sv[:, 0:2, 1:257], op=ALU.max)

            def seam_fill_from_A(src_A):
                nc.sync.dma_start(out=SEAM[0:1, 1:257],
                                  in_=src_A[127:128, 1:257])
                nc.sync.dma_start(out=SEAM[0:1, 259:515],
                                  in_=src_A[0:1, 259:515])
                seam_hmax()

            def seam_fill(src_X):
                # alpha rows 128 (g 0, t 16) and 129 (g 1, t 16) in sigma-0
                nc.sync.dma_start(
                    out=SEAM[0:1, 1:257],
                    in_=bass.AP(tensor=src_X.tensor,
                                offset=3 * FSZ + 16 * FW + 1,
                                ap=[[FSZ, 1], [1, 256]]))
                nc.sync.dma_start(
                    out=SEAM[0:1, 259:515],
                    in_=bass.AP(tensor=src_X.tensor,
                                offset=19 * FSZ + 16 * FW + 1,
                                ap=[[FSZ, 1], [1, 256]]))
                seam_hmax()

            def xwin(xt, plo, t, dj):
                # [64, 2, 256] window: partitions plo..plo+64, blocks t,t+1
                return bass.AP(tensor=xt.tensor,
                               offset=plo * FSZ + t * FW + dj,
                               ap=[[FSZ, 64], [FW, 2], [1, 256]])

            def flush_half(step, t0, t1):
                # refresh sigma copies 1..3 (or write output) for X0 blocks
                # t in [t0, t1)
                nf = (t1 - t0) * FW
                if step + 1 == steps:
                    nc.sync.dma_start(
                        out=out_ext[:, t0 * FW:t0 * FW + nf],
                        in_=X[0][:, t0 * FW:t0 * FW + nf])
                    return
                for c in range(1, NSIG):
                    npart = 128 - 16 * c
                    nc.sync.dma_start(
                        out=bass.AP(tensor=X[c].tensor,
                                    offset=16 * c * FSZ + t0 * FW,
                                    ap=[[FSZ, npart], [1, nf]]),
                        in_=bass.AP(tensor=X[0].tensor,
                                    offset=t0 * FW,
                                    ap=[[FSZ, npart], [1, nf]]))
                    nc.sync.dma_start(
                        out=bass.AP(tensor=X[c].tensor,
                                    offset=(t0 + 1) * FW,
                                    ap=[[FSZ, 16 * c], [1, nf]]),
                        in_=bass.AP(tensor=X[0].tensor,
                                    offset=npart * FSZ + t0 * FW,
                                    ap=[[FSZ, 16 * c], [1, nf]]))

            # Deferred life/flush work: closures drained a few per sweep
            # window so life-broadcast matmuls interleave with dense L1/L2
            # work (no PE-queue stalls on the single pl PSUM slot) and the
            # sigma-copy refresh DMAs overlap compute. Half-1 work of step s
            # drains during the early windows of step s+1.
            pending = []

            def drain(k):
                for _ in range(min(k, len(pending))):
                    pending.pop(0)()

            for step in range(steps):
                # --- pre pool ---
                if step == 0:
                    extract_alpha(A_pre, X[0])
                seam_fill_from_A(A_pre)
                pool_half(VMpre, A_pre, 0, HM, HMu, HMd)
                pool_half(VMpre, A_pre, 1, HM, HMu, HMd)

                d_tiles = {}
                d_count = {}
                d_expect = {tb: 8 for tb in range(NBOUT)}
                d_expect[0] = 7
                d_expect[32] = 1

                def life_block(tb, step=step):
                    lo = tb * FW + 1
                    plan = life_plan[tb]
                    plt = ph_pool.tile([128, 2, 256], F32,
                                       name=f"pl_{step}_{tb}", tag="phA")
                    pl = plt[:, 0, :]
                    for i, (half, ridx) in enumerate(plan):
                        nc.tensor.matmul(
                            pl, rt[ridx],
                            LifeF[:, half * 256:half * 256 + 256],
                            start=(i == 0), stop=(i == len(plan) - 1))
                    nc.vector.tensor_tensor(X[0][:, lo:lo + 256],
                                            XN[:, lo:lo + 256], pl,
                                            op=ALU.mult)

                def post_half(half, step=step):
                    extract_alpha(A_post, XN, halves=(half,))
                    pool_half(VMpost, A_post, half, HMp, HMpu, HMpd)
                    qlo = 256 * half
                    qs = slice(qlo, qlo + 256)
                    nc.vector.tensor_tensor(LifeF[:, qs], VMpre[:, qs],
                                            VMpost[:, qs], op=ALU.min)
                    nc.vector.tensor_scalar(LifeF[:, qs], LifeF[:, qs],
                                            0.1, None, op0=ALU.is_gt)
                    if step + 1 < steps:
                        flo = 258 * half + 1
                        nc.vector.tensor_tensor(A_pre[:, flo:flo + 256],
                                                A_post[:, flo:flo + 256],
                                                LifeF[:, qs], op=ALU.mult)
                    if half == 0:
                        for tb in range(0, 13):
                            pending.append(lambda tb=tb: life_block(tb, step))
                        pending.append(lambda: (life_block(13, step),
                                                flush_half(step, 0, 13)))
                        for tb in (14, 15):
                            pending.append(lambda tb=tb: life_block(tb, step))
                    else:
                        for tb in range(16, 21):
                            pending.append(lambda tb=tb: life_block(tb, step))
                        pending.append(lambda: flush_half(step, 13, 21))
                        for tb in range(21, 27):
                            pending.append(lambda tb=tb: life_block(tb, step))
                        pending.append(lambda: flush_half(step, 21, 27))
                        for tb in range(27, NBOUT):
                            pending.append(lambda tb=tb: life_block(tb, step))
                        pending.append(lambda: flush_half(step, 27, NB - 1))

                def l2(rho, ht, hslice):
                    tb, g = rho // 8, rho % 8
                    if tb not in d_tiles:
                        d_tiles[tb] = pd_pool.tile([128, 256], F32,
                                                   name=f"pd_s{step}_{tb}",
                                                   tag="pd")
                        d_count[tb] = 0
                    first = d_count[tb] == 0
                    d_count[tb] += 1
                    last = d_count[tb] == d_expect[tb]
                    nc.tensor.matmul(d_tiles[tb][:], w1t[g][:],
                                     ht[:, hslice], start=first, stop=last)
                    if last:
                        lo = tb * FW + 1
                        nc.vector.tensor_tensor(
                            XN[:, lo:lo + 256], d_tiles[tb][:],
                            X[0][:, lo:lo + 256], op=ALU.add)
                        if tb == 16:
                            seam_fill(XN)
                            post_half(0, step)

                # --- main sweep ---
                for M in range(0, NBOUT + 1, 2):
                    drain(3)
                    for c in range(NSIG):
                        rA = 8 * M + 1 - c
                        rB = 8 * M + 5 - c
                        ph_A = ph_pool.tile([128, 2, 256], F32, tag="phA")
                        ph_B = ph_pool.tile([128, 2, 256], F32, tag="phB")
                        for dj in range(3):
                            nc.tensor.matmul(
                                ph_A[:], LW1[0:64, 128 * dj:128 * dj + 128],
                                xwin(X[c], 0, M, dj),
                                start=(dj == 0), stop=(dj == 2))
                            nc.tensor.matmul(
                                ph_B[:], LW1[64:128, 128 * dj:128 * dj + 128],
                                xwin(X[c], 64, M, dj),
                                start=(dj == 0), stop=(dj == 2))
                        ht_A = hpool.tile([128, 512], F16, tag="htA")
                        relu_tile(ht_A[:], ph_A.rearrange("p a b -> p (a b)"))
                        ht_B = hpool.tile([128, 512], F16, tag="htB")
                        relu_tile(ht_B[:], ph_B.rearrange("p a b -> p (a b)"))
                        for r0, ht in ((rA, ht_A), (rB, ht_B)):
                            if 1 <= r0 <= 256:
                                l2(r0, ht, slice(0, 256))
                            if 1 <= r0 + 8 <= 256:
                                l2(r0 + 8, ht, slice(256, 512))

                post_half(1, step)
                if step + 1 == steps:
                    drain(len(pending))

    nc.compile()
    return nc


_PROGRAM_CACHE = {}


def kernel(x, W0, b0, W1, steps, _trace=False):
    import concourse.bass_utils as bass_utils
    steps = int(steps)
    x = np.asarray(x, dtype=np.float32)
    W0 = np.asarray(W0, dtype=np.float32)
    b0 = np.asarray(b0, dtype=np.float32)
    W1 = np.asarray(W1, dtype=np.float32)
    B = x.shape[0]
    assert x.shape == (8, H, W, C), x.shape

    wts = build_weights(W0, b0, W1)
    key = steps
    if key not in _PROGRAM_CACHE:
        _PROGRAM_CACHE[key] = build_program(steps, wts["life_plan"],
                                            wts["r_stack"].shape[0])
    nc = _PROGRAM_CACHE[key]

    in_maps = []
    for b in range(B):
        xcs = marshal_x(x[b])
        m = {f"xc{c}": xcs[c] for c in range(NSIG)}
        m.update({
            "l1w": wts["l1w"],
            "w1w": wts["w1_stack"],
            "rw": wts["r_stack"],
            "b0w": wts["b0"],
        })
        in_maps.append(m)
    res = bass_utils.run_bass_kernel_spmd(nc, in_maps, list(range(8)),
                                          trace=_trace)
    kernel.last_result = res
    out = np.stack([unmarshal_x(res.results[b]["out"]) for b in range(B)])
    return out.astype(np.float32)


# revision 8
# speedup vs baseline: 1.1808x; 1.1808x over previous
"""Trainium2 Bass kernel for nn_CA_Model (neural cellular automaton).

Data-parallel over batch (8 images -> 8 cores). Per core the [256,256,16]
image lives in SBUF as FOUR fp16 row-shifted copies
  Xc[16*((r+c)%8)+ch, ((r+c)//8)*258 + 1 + w],  c in 0..3
so that every output row pair (rho, rho+4) finds its 3-row conv support on
partition strips {0,1} (group 1) and {2,3} (group 5) of the SAME copy:
the three dj matmuls of the 3x3-perceive + layer-1 fuse run as K=64
tile_position pairs that execute concurrently on disjoint PE row strips.

Layer 2 accumulates eight per-row matmuls (zero-padded W1^T columns) into a
PSUM tile per row-block. Alive maxpools run on a q2-interleaved alpha tile;
vertical max via DMA partition-shifted copies. State updates write the sigma=0
copy; copies 1..3 are refreshed by two batched partition-rotating DMAs per
copy per image half, overlapped with compute of the other half.
"""
import sys
for _p in ("/opt/trn_rl_repo", "/root/.axon_site/_ro/trn_rl_repo"):
    if _p not in sys.path:
        sys.path.append(_p)

import numpy as np

C = 16
HID = 128
H = W = 256
NB = 34            # row blocks in layout (34*8 = 272 slots; rows 0..257+shift used)
NBOUT = 33         # blocks that can hold image rows in sigma-0 layout
FW = 258           # padded row width in free dim
FSZ = NB * FW      # 8772 free elements per partition
NSIG = 4           # shifted copies


def _sobel():
    dx = np.outer([1, 2, 1], [-1, 0, 1]) / 8.0
    f1 = dx.T.astype(np.float32)
    f2 = dx.astype(np.float32)
    return f1, f2


def build_weights(W0, b0, W1):
    import ml_dtypes
    F1, F2 = _sobel()
    W0x, W0y1, W0y2 = W0[:, 0:16], W0[:, 16:32], W0[:, 32:48]
    # A[di][dj]: [HID, C] applied to x[row-1+di, w-1+dj]
    A = [[(np.float32(di == 1 and dj == 1) * W0x
           + F1[di, dj] * W0y1 + F2[di, dj] * W0y2).astype(np.float32)
          for dj in range(3)] for di in range(3)]

    # L1W [128, 3*128]: column block dj holds the K=64 lhsT for the A side
    # (support groups 0,1,2 -> partitions 16*di+c) and, shifted down 64, the
    # B side (groups 4,5,6 -> partitions 64+16*di+c).
    L1W = np.zeros((128, 3 * 128), np.float32)
    for dj in range(3):
        for di in range(3):
            L1W[16 * di:16 * di + 16, 128 * dj:128 * dj + 128] = A[di][dj].T
            L1W[64 + 16 * di:64 + 16 * di + 16, 128 * dj:128 * dj + 128] = A[di][dj].T

    # layer-2 lhsT: for a row with group g, W1pad[g][n, 16g+c] = W1[c, n]
    w1p = []
    for g in range(8):
        Wp = np.zeros((128, 128), np.float32)
        Wp[:, 16 * g:16 * g + 16] = W1.T
        w1p.append(Wp)

    # life-broadcast lhsT variants (q2 layout of LifeF -> X layout), per tb
    life_plan = []
    r_mats = []
    for tb in range(NBOUT):
        plan = []
        buckets = {}
        for g in range(8):
            rho = 8 * tb + g
            if rho < 1 or rho > 256:
                continue
            q = rho - 1
            half, qh = q // 128, q % 128
            buckets.setdefault(half, []).append((g, qh))
        for half, rows in sorted(buckets.items()):
            Rm = np.zeros((128, 128), np.float32)
            for g, qh in rows:
                q2 = (qh % 8) * 16 + qh // 8
                Rm[q2, 16 * g:16 * g + 16] = 1.0
            plan.append((half, len(r_mats)))
            r_mats.append(Rm)
        life_plan.append(plan)

    f16 = np.float16
    return dict(l1w=L1W.astype(f16),
                w1_stack=np.stack(w1p).astype(__import__('ml_dtypes').bfloat16),
                r_stack=np.stack(r_mats).astype(f16),
                life_plan=life_plan,
                b0=b0.reshape(128, 1).astype(np.float32))


def marshal_x(img):
    """[256,256,16] image -> 4 shifted copies [128, FSZ] fp16."""
    out = []
    for c in range(NSIG):
        xp = np.zeros((NB * 8, FW, C), np.float16)
        xp[1 + c:257 + c, 1:257, :] = img
        xc = xp.reshape(NB, 8, FW, C).transpose(1, 3, 0, 2).reshape(128, FSZ)
        out.append(np.ascontiguousarray(xc))
    return out


def unmarshal_x(xc):
    """X0 [128, FSZ] -> [256,256,16] image."""
    xp = xc.reshape(8, C, NB, FW).transpose(2, 0, 3, 1)
    xp = xp.reshape(NB * 8, FW, C)
    return np.ascontiguousarray(xp[1:257, 1:257, :])


def build_program(steps, life_plan, n_r,
                  relu_split=(6, 2, 0), debug_phases=99):
    """relu_split: of every sum(relu_split) relu tiles, how many go to
    (scalar, vector, gpsimd)."""
    import concourse.bass as bass
    import concourse.bacc as bacc
    import concourse.tile as tile
    from concourse import mybir
    F32 = mybir.dt.float32
    F16 = mybir.dt.float16
    BF16 = mybir.dt.bfloat16
    AF = mybir.ActivationFunctionType
    ALU = mybir.AluOpType
    nc = bacc.Bacc(None, target_bir_lowering=False, debug=False, num_devices=8,
                   num_swdge_queues=4)

    x_ext = [nc.declare_dram_parameter(f"xc{c}", [128, FSZ], F16, isOutput=False)
             for c in range(NSIG)]
    l1_ext = nc.declare_dram_parameter("l1w", [128, 3 * 128], F16, isOutput=False)
    w1_ext = nc.declare_dram_parameter("w1w", [8, 128, 128], BF16, isOutput=False)
    r_ext = nc.declare_dram_parameter("rw", [n_r, 128, 128], F16, isOutput=False)
    b0_ext = nc.declare_dram_parameter("b0w", [128, 1], F32, isOutput=False)
    out_ext = nc.declare_dram_parameter("out", [128, FSZ], F16, isOutput=True)

    with tile.TileContext(nc) as tc:
        with tc.tile_pool(name="hpool", bufs=4) as hpool, \
             tc.tile_pool(name="ph_pool", bufs=2, space="PSUM") as ph_pool, \
             tc.tile_pool(name="pd_pool", bufs=3, space="PSUM") as pd_pool:

            # --- persistent state ---
            X = [nc.alloc_sbuf_tensor(f"X{c}", [128, FSZ], F16).ap()
                 for c in range(NSIG)]
            XN = nc.alloc_sbuf_tensor("XN", [128, FSZ], F16).ap()

            LW1 = nc.alloc_sbuf_tensor("LW1", [128, 3 * 128], F16).ap()
            LW2 = nc.alloc_sbuf_tensor("LW2", [128, 8 * 128], BF16).ap()
            LWR = nc.alloc_sbuf_tensor("LWR", [128, n_r * 128], F16).ap()
            w1t = [LW2[:, 128 * g:128 * g + 128] for g in range(8)]
            rt = [LWR[:, 128 * i:128 * i + 128] for i in range(n_r)]
            b0t = nc.alloc_sbuf_tensor("b0t", [128, 1], F32).ap()

            A_pre = nc.alloc_sbuf_tensor("A_pre", [128, 516], F16).ap()
            A_post = nc.alloc_sbuf_tensor("A_post", [128, 516], F16).ap()
            HM = nc.alloc_sbuf_tensor("HM", [128, 512], F16).ap()
            HMu = nc.alloc_sbuf_tensor("HMu", [128, 512], F16).ap()
            HMd = nc.alloc_sbuf_tensor("HMd", [128, 512], F16).ap()
            HMp = nc.alloc_sbuf_tensor("HMp", [128, 512], F16).ap()
            HMpu = nc.alloc_sbuf_tensor("HMpu", [128, 512], F16).ap()
            HMpd = nc.alloc_sbuf_tensor("HMpd", [128, 512], F16).ap()
            # seam scratch: partition 0: [0:258] a128 | [258:516] a129 |
            # [516:774] hm128 | [774:1032] hm129
            SEAM = nc.alloc_sbuf_tensor("SEAM", [128, 1032], F16).ap()
            VMpre = nc.alloc_sbuf_tensor("VMpre", [128, 512], F16).ap()
            VMpost = nc.alloc_sbuf_tensor("VMpost", [128, 512], F16).ap()
            LifeF = nc.alloc_sbuf_tensor("LifeF", [128, 512], F16).ap()
            Zrow = nc.alloc_sbuf_tensor("Zrow", [128, 516], F16).ap()

            # --- loads / init ---
            for c in range(NSIG):
                nc.gpsimd.dma_start(out=X[c][:], in_=x_ext[c][:])
            nc.gpsimd.dma_start(out=LW1[:], in_=l1_ext[:])
            nc.gpsimd.dma_start(out=LW2[:], in_=bass.AP(
                tensor=w1_ext, offset=0,
                ap=[[128, 128], [128 * 128, 8], [1, 128]]))
            nc.gpsimd.dma_start(out=LWR[:], in_=bass.AP(
                tensor=r_ext, offset=0,
                ap=[[128, 128], [128 * 128, n_r], [1, 128]]))
            nc.gpsimd.dma_start(out=b0t[:], in_=b0_ext[:])
            nc.vector.memset(Zrow[:], 0.0)
            nc.vector.memset(SEAM[0:32, :], 0.0)
            nc.vector.memset(A_post[:], 0.0)
            nc.vector.memset(A_pre[:], 0.0)

            relu_ctr = [0]
            rs_total = sum(relu_split)
            rs_s, rs_v = relu_split[0], relu_split[0] + relu_split[1]

            def relu_tile(dst, src):
                k = relu_ctr[0] % rs_total
                relu_ctr[0] += 1
                if k < rs_s:
                    nc.scalar.activation(dst, src, AF.Relu, bias=b0t[:], scale=1.0)
                elif k < rs_v:
                    nc.vector.tensor_scalar(dst, src, b0t[:], 0.0,
                                            op0=ALU.add, op1=ALU.max)
                else:
                    nc.vector.tensor_scalar(dst, src, b0t[:], 0.0,
                                            op0=ALU.add, op1=ALU.max)

            def extract_alpha(dst_A, src_X, halves=(0, 1)):
                # q2-layout: dst_A[gp*16 + j, half*258 + 1 + w] holds alpha of
                # row rho = 128*half + 8j + gp + 1 (alpha: channel 3).
                for half in halves:
                    for gp in range(8):
                        g = (gp + 1) % 8
                        t0 = 16 * half + (1 if gp == 7 else 0)
                        dst = bass.AP(
                            tensor=dst_A.tensor,
                            offset=16 * gp * 516 + 258 * half + 1,
                            ap=[[516, 16], [1, 256]])
                        srcap = bass.AP(
                            tensor=src_X.tensor,
                            offset=(16 * g + 3) * FSZ + t0 * FW + 1,
                            ap=[[FSZ, 1], [FW, 16], [1, 256]])
                        eng = nc.sync if gp % 2 == 0 else nc.gpsimd
                        eng.dma_start(out=dst, in_=srcap)

            def pool_half(dst_VM, src_A, half, hm, hmu, hmd):
                lo, hi = 258 * half, 258 * half + 258
                qlo, qhi = 256 * half, 256 * half + 256
                av = src_A[:, lo:hi]
                nc.vector.tensor_tensor(hm[:, qlo:qhi], av[:, 0:256],
                                        av[:, 2:258], op=ALU.max)
                nc.vector.tensor_tensor(hm[:, qlo:qhi], hm[:, qlo:qhi],
                                        av[:, 1:257], op=ALU.max)
                nc.gpsimd.dma_start(out=hmu[0:112, qlo:qhi], in_=hm[16:128, qlo:qhi])
                nc.sync.dma_start(out=hmu[112:127, qlo:qhi], in_=hm[1:16, qlo:qhi])
                nc.gpsimd.dma_start(out=hmd[16:128, qlo:qhi], in_=hm[0:112, qlo:qhi])
                nc.sync.dma_start(out=hmd[1:16, qlo:qhi], in_=hm[112:127, qlo:qhi])
                if half == 0:
                    nc.gpsimd.dma_start(out=hmu[127:128, 0:256],
                                        in_=SEAM[0:1, 775:1031])
                    nc.sync.dma_start(out=hmd[0:1, 0:256], in_=Zrow[0:1, 0:256])
                else:
                    nc.gpsimd.dma_start(out=hmd[0:1, 256:512],
                                        in_=SEAM[0:1, 517:773])
                    nc.sync.dma_start(out=hmu[127:128, 256:512],
                                      in_=Zrow[0:1, 0:256])
                nc.vector.tensor_tensor(dst_VM[:, qlo:qhi], hm[:, qlo:qhi],
                                        hmu[:, qlo:qhi], op=ALU.max)
                nc.vector.tensor_tensor(dst_VM[:, qlo:qhi], dst_VM[:, qlo:qhi],
                                        hmd[:, qlo:qhi], op=ALU.max)

            def seam_hmax():
                sv = SEAM[0:1, :].rearrange("p (a w) -> p a w", a=4)
                nc.vector.tensor_tensor(sv[:, 2:4, 1:257], sv[:, 0:2, 0:256],
                                        sv[:, 0:2, 2:258], op=ALU.max)
                nc.vector.tensor_tensor(sv[:, 2:4, 1:257], sv[:, 2:4, 1:257],
                                        sv[:, 0:2, 1:257], op=ALU.max)

            def seam_fill_from_A(src_A):
                nc.sync.dma_start(out=SEAM[0:1, 1:257],
                                  in_=src_A[127:128, 1:257])
                nc.sync.dma_start(out=SEAM[0:1, 259:515],
                                  in_=src_A[0:1, 259:515])
                seam_hmax()

            def seam_fill(src_X):
                # alpha rows 128 (g 0, t 16) and 129 (g 1, t 16) in sigma-0
                nc.sync.dma_start(
                    out=SEAM[0:1, 1:257],
                    in_=bass.AP(tensor=src_X.tensor,
                                offset=3 * FSZ + 16 * FW + 1,
                                ap=[[FSZ, 1], [1, 256]]))
                nc.sync.dma_start(
                    out=SEAM[0:1, 259:515],
                    in_=bass.AP(tensor=src_X.tensor,
                                offset=19 * FSZ + 16 * FW + 1,
                                ap=[[FSZ, 1], [1, 256]]))
                seam_hmax()

            def xwin(xt, plo, t, dj):
                # [64, 2, 256] window: partitions plo..plo+64, blocks t,t+1
                return bass.AP(tensor=xt.tensor,
                               offset=plo * FSZ + t * FW + dj,
                               ap=[[FSZ, 64], [FW, 2], [1, 256]])

            def flush_half(step, t0, t1):
                # refresh sigma copies 1..3 (or write output) for X0 blocks
                # t in [t0, t1)
                nf = (t1 - t0) * FW
                if step + 1 == steps:
                    nc.sync.dma_start(
                        out=out_ext[:, t0 * FW:t0 * FW + nf],
                        in_=X[0][:, t0 * FW:t0 * FW + nf])
                    return
                for c in range(1, NSIG):
                    npart = 128 - 16 * c
                    nc.gpsimd.dma_start(
                        out=bass.AP(tensor=X[c].tensor,
                                    offset=16 * c * FSZ + t0 * FW,
                                    ap=[[FSZ, npart], [1, nf]]),
                        in_=bass.AP(tensor=X[0].tensor,
                                    offset=t0 * FW,
                                    ap=[[FSZ, npart], [1, nf]]))
                    nc.sync.dma_start(
                        out=bass.AP(tensor=X[c].tensor,
                                    offset=(t0 + 1) * FW,
                                    ap=[[FSZ, 16 * c], [1, nf]]),
                        in_=bass.AP(tensor=X[0].tensor,
                                    offset=npart * FSZ + t0 * FW,
                                    ap=[[FSZ, 16 * c], [1, nf]]))

            # Deferred life/flush work: closures drained a few per sweep
            # window so life-broadcast matmuls interleave with dense L1/L2
            # work (no PE-queue stalls on the single pl PSUM slot) and the
            # sigma-copy refresh DMAs overlap compute. Half-1 work of step s
            # drains during the early windows of step s+1.
            pending = []

            def drain(k):
                for _ in range(min(k, len(pending))):
                    pending.pop(0)()

            for step in range(steps):
                # --- pre pool ---
                if step == 0:
                    extract_alpha(A_pre, X[0])
                seam_fill_from_A(A_pre)
                pool_half(VMpre, A_pre, 0, HM, HMu, HMd)
                pool_half(VMpre, A_pre, 1, HM, HMu, HMd)

                d_tiles = {}
                d_count = {}
                d_expect = {tb: 8 for tb in range(NBOUT)}
                d_expect[0] = 7
                d_expect[32] = 1

                def life_block(tb, step=step):
                    lo = tb * FW + 1
                    plan = life_plan[tb]
                    plt = ph_pool.tile([128, 2, 256], F32,
                                       name=f"pl_{step}_{tb}", tag="phA")
                    pl = plt[:, 0, :]
                    for i, (half, ridx) in enumerate(plan):
                        nc.tensor.matmul(
                            pl, rt[ridx],
                            LifeF[:, half * 256:half * 256 + 256],
                            start=(i == 0), stop=(i == len(plan) - 1))
                    nc.vector.tensor_tensor(X[0][:, lo:lo + 256],
                                            XN[:, lo:lo + 256], pl,
                                            op=ALU.mult)

                def post_half(half, step=step):
                    extract_alpha(A_post, XN, halves=(half,))
                    pool_half(VMpost, A_post, half, HMp, HMpu, HMpd)
                    qlo = 256 * half
                    qs = slice(qlo, qlo + 256)
                    nc.vector.tensor_tensor(LifeF[:, qs], VMpre[:, qs],
                                            VMpost[:, qs], op=ALU.min)
                    nc.vector.tensor_scalar(LifeF[:, qs], LifeF[:, qs],
                                            0.1, None, op0=ALU.is_gt)
                    if step + 1 < steps:
                        flo = 258 * half + 1
                        nc.vector.tensor_tensor(A_pre[:, flo:flo + 256],
                                                A_post[:, flo:flo + 256],
                                                LifeF[:, qs], op=ALU.mult)
                    if half == 0:
                        for tb in range(0, 13):
                            pending.append(lambda tb=tb: life_block(tb, step))
                        pending.append(lambda: (life_block(13, step),
                                                flush_half(step, 0, 13)))
                        for tb in (14, 15):
                            pending.append(lambda tb=tb: life_block(tb, step))
                    else:
                        for tb in range(16, 21):
                            pending.append(lambda tb=tb: life_block(tb, step))
                        pending.append(lambda: flush_half(step, 13, 21))
                        for tb in range(21, 27):
                            pending.append(lambda tb=tb: life_block(tb, step))
                        pending.append(lambda: flush_half(step, 21, 27))
                        for tb in range(27, NBOUT):
                            pending.append(lambda tb=tb: life_block(tb, step))
                        pending.append(lambda: flush_half(step, 27, NB - 1))

                def l2(rho, ht, hslice):
                    tb, g = rho // 8, rho % 8
                    if tb not in d_tiles:
                        d_tiles[tb] = pd_pool.tile([128, 256], F32,
                                                   name=f"pd_s{step}_{tb}",
                                                   tag="pd")
                        d_count[tb] = 0
                    first = d_count[tb] == 0
                    d_count[tb] += 1
                    last = d_count[tb] == d_expect[tb]
                    nc.tensor.matmul(d_tiles[tb][:], w1t[g][:],
                                     ht[:, hslice], start=first, stop=last)
                    if last:
                        lo = tb * FW + 1
                        nc.vector.tensor_tensor(
                            XN[:, lo:lo + 256], d_tiles[tb][:],
                            X[0][:, lo:lo + 256], op=ALU.add)
                        if tb == 16:
                            seam_fill(XN)
                            post_half(0, step)

                # --- main sweep ---
                for M in range(0, NBOUT + 1, 2):
                    drain(3)
                    for c in range(NSIG):
                        rA = 8 * M + 1 - c
                        rB = 8 * M + 5 - c
                        ph_A = ph_pool.tile([128, 2, 256], F32, tag="phA")
                        ph_B = ph_pool.tile([128, 2, 256], F32, tag="phB")
                        for dj in range(3):
                            nc.tensor.matmul(
                                ph_A[:], LW1[0:64, 128 * dj:128 * dj + 128],
                                xwin(X[c], 0, M, dj),
                                start=(dj == 0), stop=(dj == 2))
                            nc.tensor.matmul(
                                ph_B[:], LW1[64:128, 128 * dj:128 * dj + 128],
                                xwin(X[c], 64, M, dj),
                                start=(dj == 0), stop=(dj == 2))
                        ht_A = hpool.tile([128, 512], BF16, tag="htA")
                        relu_tile(ht_A[:], ph_A.rearrange("p a b -> p (a b)"))
                        ht_B = hpool.tile([128, 512], BF16, tag="htB")
                        relu_tile(ht_B[:], ph_B.rearrange("p a b -> p (a b)"))
                        for r0, ht in ((rA, ht_A), (rB, ht_B)):
                            if 1 <= r0 <= 256:
                                l2(r0, ht, slice(0, 256))
                            if 1 <= r0 + 8 <= 256:
                                l2(r0 + 8, ht, slice(256, 512))

                post_half(1, step)
                if step + 1 == steps:
                    drain(len(pending))

    nc.compile()
    return nc


_PROGRAM_CACHE = {}


def kernel(x, W0, b0, W1, steps, _trace=False):
    import concourse.bass_utils as bass_utils
    steps = int(steps)
    x = np.asarray(x, dtype=np.float32)
    W0 = np.asarray(W0, dtype=np.float32)
    b0 = np.asarray(b0, dtype=np.float32)
    W1 = np.asarray(W1, dtype=np.float32)
    B = x.shape[0]
    assert x.shape == (8, H, W, C), x.shape

    wts = build_weights(W0, b0, W1)
    key = steps
    if key not in _PROGRAM_CACHE:
        _PROGRAM_CACHE[key] = build_program(steps, wts["life_plan"],
                                            wts["r_stack"].shape[0])
    nc = _PROGRAM_CACHE[key]

    in_maps = []
    for b in range(B):
        xcs = marshal_x(x[b])
        m = {f"xc{c}": xcs[c] for c in range(NSIG)}
        m.update({
            "l1w": wts["l1w"],
            "w1w": wts["w1_stack"],
            "rw": wts["r_stack"],
            "b0w": wts["b0"],
        })
        in_maps.append(m)
    res = bass_utils.run_bass_kernel_spmd(nc, in_maps, list(range(8)),
                                          trace=_trace)
    kernel.last_result = res
    out = np.stack([unmarshal_x(res.results[b]["out"]) for b in range(B)])
    return out.astype(np.float32)


# revision 9
# speedup vs baseline: 1.3352x; 1.1308x over previous
"""Trainium2 Bass kernel for nn_CA_Model (neural cellular automaton).

Data-parallel over batch (8 images -> 8 cores). Per core the [256,256,16]
image lives in SBUF as FOUR fp16 row-shifted copies
  Xc[16*((r+c)%8)+ch, ((r+c)//8)*258 + 1 + w],  c in 0..3
so that every output row pair (rho, rho+4) finds its 3-row conv support on
partition strips {0,1} (group 1) and {2,3} (group 5) of the SAME copy:
the three dj matmuls of the 3x3-perceive + layer-1 fuse run as K=64
tile_position pairs that execute concurrently on disjoint PE row strips.

Layer 2 accumulates eight per-row matmuls (zero-padded W1^T columns) into a
PSUM tile per row-block. Alive maxpools run on a q2-interleaved alpha tile;
vertical max via DMA partition-shifted copies. State updates write the sigma=0
copy; copies 1..3 are refreshed by two batched partition-rotating DMAs per
copy per image half, overlapped with compute of the other half.
"""
import sys
for _p in ("/opt/trn_rl_repo", "/root/.axon_site/_ro/trn_rl_repo"):
    if _p not in sys.path:
        sys.path.append(_p)

import numpy as np

C = 16
HID = 128
H = W = 256
NB = 34            # row blocks in layout (34*8 = 272 slots; rows 0..257+shift used)
NBOUT = 33         # blocks that can hold image rows in sigma-0 layout
FW = 258           # padded row width in free dim
FSZ = NB * FW      # 8772 free elements per partition
NSIG = 4           # shifted copies


def _sobel():
    dx = np.outer([1, 2, 1], [-1, 0, 1]) / 8.0
    f1 = dx.T.astype(np.float32)
    f2 = dx.astype(np.float32)
    return f1, f2


def build_weights(W0, b0, W1):
    import ml_dtypes
    F1, F2 = _sobel()
    W0x, W0y1, W0y2 = W0[:, 0:16], W0[:, 16:32], W0[:, 32:48]
    # A[di][dj]: [HID, C] applied to x[row-1+di, w-1+dj]
    A = [[(np.float32(di == 1 and dj == 1) * W0x
           + F1[di, dj] * W0y1 + F2[di, dj] * W0y2).astype(np.float32)
          for dj in range(3)] for di in range(3)]

    # L1W [128, 3*128]: column block dj holds the K=64 lhsT for the A side
    # (support groups 0,1,2 -> partitions 16*di+c) and, shifted down 64, the
    # B side (groups 4,5,6 -> partitions 64+16*di+c).
    L1W = np.zeros((128, 3 * 128), np.float32)
    for dj in range(3):
        for di in range(3):
            L1W[16 * di:16 * di + 16, 128 * dj:128 * dj + 128] = A[di][dj].T
            L1W[64 + 16 * di:64 + 16 * di + 16, 128 * dj:128 * dj + 128] = A[di][dj].T

    # layer-2 lhsT: for a row with group g, W1pad[g][n, 16g+c] = W1[c, n]
    w1p = []
    for g in range(8):
        Wp = np.zeros((128, 128), np.float32)
        Wp[:, 16 * g:16 * g + 16] = W1.T
        w1p.append(Wp)

    # life-broadcast lhsT variants (q2 layout of LifeF -> X layout), per tb
    life_plan = []
    r_mats = []
    for tb in range(NBOUT):
        plan = []
        buckets = {}
        for g in range(8):
            rho = 8 * tb + g
            if rho < 1 or rho > 256:
                continue
            q = rho - 1
            half, qh = q // 128, q % 128
            buckets.setdefault(half, []).append((g, qh))
        for half, rows in sorted(buckets.items()):
            Rm = np.zeros((128, 128), np.float32)
            for g, qh in rows:
                q2 = (qh % 8) * 16 + qh // 8
                Rm[q2, 16 * g:16 * g + 16] = 1.0
            plan.append((half, len(r_mats)))
            r_mats.append(Rm)
        life_plan.append(plan)

    f16 = np.float16
    return dict(l1w=L1W.astype(f16),
                w1_stack=np.stack(w1p).astype(__import__('ml_dtypes').bfloat16),
                r_stack=np.stack(r_mats).astype(f16),
                life_plan=life_plan,
                b0=b0.reshape(128, 1).astype(np.float32))


def marshal_x(img):
    """[256,256,16] image -> 4 shifted copies [128, FSZ] fp16."""
    out = []
    for c in range(NSIG):
        xp = np.zeros((NB * 8, FW, C), np.float16)
        xp[1 + c:257 + c, 1:257, :] = img
        xc = xp.reshape(NB, 8, FW, C).transpose(1, 3, 0, 2).reshape(128, FSZ)
        out.append(np.ascontiguousarray(xc))
    return out


def unmarshal_x(xc):
    """X0 [128, FSZ] -> [256,256,16] image."""
    xp = xc.reshape(8, C, NB, FW).transpose(2, 0, 3, 1)
    xp = xp.reshape(NB * 8, FW, C)
    return np.ascontiguousarray(xp[1:257, 1:257, :])


def build_program(steps, life_plan, n_r,
                  relu_split=(6, 2, 0), debug_phases=99):
    """relu_split: of every sum(relu_split) relu tiles, how many go to
    (scalar, vector, gpsimd)."""
    import concourse.bass as bass
    import concourse.bacc as bacc
    import concourse.tile as tile
    from concourse import mybir
    F32 = mybir.dt.float32
    F16 = mybir.dt.float16
    BF16 = mybir.dt.bfloat16
    AF = mybir.ActivationFunctionType
    ALU = mybir.AluOpType
    nc = bacc.Bacc(None, target_bir_lowering=False, debug=False, num_devices=8,
                   num_swdge_queues=4)

    x_ext = [nc.declare_dram_parameter(f"xc{c}", [128, FSZ], F16, isOutput=False)
             for c in range(NSIG)]
    l1_ext = nc.declare_dram_parameter("l1w", [128, 3 * 128], F16, isOutput=False)
    w1_ext = nc.declare_dram_parameter("w1w", [8, 128, 128], BF16, isOutput=False)
    r_ext = nc.declare_dram_parameter("rw", [n_r, 128, 128], F16, isOutput=False)
    b0_ext = nc.declare_dram_parameter("b0w", [128, 1], F32, isOutput=False)
    out_ext = nc.declare_dram_parameter("out", [128, FSZ], F16, isOutput=True)

    with tile.TileContext(nc) as tc:
        with tc.tile_pool(name="hpool", bufs=4) as hpool, \
             tc.tile_pool(name="ph_pool", bufs=2, space="PSUM") as ph_pool, \
             tc.tile_pool(name="pd_pool", bufs=3, space="PSUM") as pd_pool:

            # --- persistent state ---
            X = [nc.alloc_sbuf_tensor(f"X{c}", [128, FSZ], F16).ap()
                 for c in range(NSIG)]
            XN = nc.alloc_sbuf_tensor("XN", [128, FSZ], F16).ap()

            LW1 = nc.alloc_sbuf_tensor("LW1", [128, 3 * 128], F16).ap()
            LW2 = nc.alloc_sbuf_tensor("LW2", [128, 8 * 128], BF16).ap()
            LWR = nc.alloc_sbuf_tensor("LWR", [128, n_r * 128], F16).ap()
            w1t = [LW2[:, 128 * g:128 * g + 128] for g in range(8)]
            rt = [LWR[:, 128 * i:128 * i + 128] for i in range(n_r)]
            b0t = nc.alloc_sbuf_tensor("b0t", [128, 1], F32).ap()

            A_pre = nc.alloc_sbuf_tensor("A_pre", [128, 516], F16).ap()
            A_post = nc.alloc_sbuf_tensor("A_post", [128, 516], F16).ap()
            HM = nc.alloc_sbuf_tensor("HM", [128, 512], F16).ap()
            HMu = nc.alloc_sbuf_tensor("HMu", [128, 512], F16).ap()
            HMd = nc.alloc_sbuf_tensor("HMd", [128, 512], F16).ap()
            HMp = nc.alloc_sbuf_tensor("HMp", [128, 512], F16).ap()
            HMpu = nc.alloc_sbuf_tensor("HMpu", [128, 512], F16).ap()
            HMpd = nc.alloc_sbuf_tensor("HMpd", [128, 512], F16).ap()
            # seam scratch: partition 0: [0:258] a128 | [258:516] a129 |
            # [516:774] hm128 | [774:1032] hm129
            SEAM = nc.alloc_sbuf_tensor("SEAM", [128, 1032], F16).ap()
            VMpre = nc.alloc_sbuf_tensor("VMpre", [128, 512], F16).ap()
            VMpost = nc.alloc_sbuf_tensor("VMpost", [128, 512], F16).ap()
            LifeF = nc.alloc_sbuf_tensor("LifeF", [128, 512], F16).ap()
            Zrow = nc.alloc_sbuf_tensor("Zrow", [128, 516], F16).ap()

            # --- loads / init ---
            for c in range(NSIG):
                nc.gpsimd.dma_start(out=X[c][:], in_=x_ext[c][:])
            nc.gpsimd.dma_start(out=LW1[:], in_=l1_ext[:])
            nc.gpsimd.dma_start(out=LW2[:], in_=bass.AP(
                tensor=w1_ext, offset=0,
                ap=[[128, 128], [128 * 128, 8], [1, 128]]))
            nc.gpsimd.dma_start(out=LWR[:], in_=bass.AP(
                tensor=r_ext, offset=0,
                ap=[[128, 128], [128 * 128, n_r], [1, 128]]))
            nc.gpsimd.dma_start(out=b0t[:], in_=b0_ext[:])
            nc.vector.memset(Zrow[:], 0.0)
            nc.vector.memset(SEAM[0:32, :], 0.0)
            nc.vector.memset(A_post[:], 0.0)
            nc.vector.memset(A_pre[:], 0.0)

            relu_ctr = [0]
            rs_total = sum(relu_split)
            rs_s, rs_v = relu_split[0], relu_split[0] + relu_split[1]

            def relu_tile(dst, src):
                k = relu_ctr[0] % rs_total
                relu_ctr[0] += 1
                if k < rs_s:
                    nc.scalar.activation(dst, src, AF.Relu, bias=b0t[:], scale=1.0)
                elif k < rs_v:
                    nc.vector.tensor_scalar(dst, src, b0t[:], 0.0,
                                            op0=ALU.add, op1=ALU.max)
                else:
                    nc.vector.tensor_scalar(dst, src, b0t[:], 0.0,
                                            op0=ALU.add, op1=ALU.max)

            def extract_alpha(dst_A, src_X, halves=(0, 1)):
                # q2-layout: dst_A[gp*16 + j, half*258 + 1 + w] holds alpha of
                # row rho = 128*half + 8j + gp + 1 (alpha: channel 3).
                for half in halves:
                    for gp in range(8):
                        g = (gp + 1) % 8
                        t0 = 16 * half + (1 if gp == 7 else 0)
                        dst = bass.AP(
                            tensor=dst_A.tensor,
                            offset=16 * gp * 516 + 258 * half + 1,
                            ap=[[516, 16], [1, 256]])
                        srcap = bass.AP(
                            tensor=src_X.tensor,
                            offset=(16 * g + 3) * FSZ + t0 * FW + 1,
                            ap=[[FSZ, 1], [FW, 16], [1, 256]])
                        eng = nc.sync if gp % 2 == 0 else nc.gpsimd
                        eng.dma_start(out=dst, in_=srcap)

            def pool_half(dst_VM, src_A, half, hm, hmu, hmd):
                lo, hi = 258 * half, 258 * half + 258
                qlo, qhi = 256 * half, 256 * half + 256
                av = src_A[:, lo:hi]
                nc.vector.tensor_tensor(hm[:, qlo:qhi], av[:, 0:256],
                                        av[:, 2:258], op=ALU.max)
                nc.vector.tensor_tensor(hm[:, qlo:qhi], hm[:, qlo:qhi],
                                        av[:, 1:257], op=ALU.max)
                nc.gpsimd.dma_start(out=hmu[0:112, qlo:qhi], in_=hm[16:128, qlo:qhi])
                nc.sync.dma_start(out=hmu[112:127, qlo:qhi], in_=hm[1:16, qlo:qhi])
                nc.gpsimd.dma_start(out=hmd[16:128, qlo:qhi], in_=hm[0:112, qlo:qhi])
                nc.sync.dma_start(out=hmd[1:16, qlo:qhi], in_=hm[112:127, qlo:qhi])
                if half == 0:
                    nc.gpsimd.dma_start(out=hmu[127:128, 0:256],
                                        in_=SEAM[0:1, 775:1031])
                    nc.sync.dma_start(out=hmd[0:1, 0:256], in_=Zrow[0:1, 0:256])
                else:
                    nc.gpsimd.dma_start(out=hmd[0:1, 256:512],
                                        in_=SEAM[0:1, 517:773])
                    nc.sync.dma_start(out=hmu[127:128, 256:512],
                                      in_=Zrow[0:1, 0:256])
                nc.vector.tensor_tensor(dst_VM[:, qlo:qhi], hm[:, qlo:qhi],
                                        hmu[:, qlo:qhi], op=ALU.max)
                nc.vector.tensor_tensor(dst_VM[:, qlo:qhi], dst_VM[:, qlo:qhi],
                                        hmd[:, qlo:qhi], op=ALU.max)

            def seam_hmax():
                sv = SEAM[0:1, :].rearrange("p (a w) -> p a w", a=4)
                nc.vector.tensor_tensor(sv[:, 2:4, 1:257], sv[:, 0:2, 0:256],
                                        sv[:, 0:2, 2:258], op=ALU.max)
                nc.vector.tensor_tensor(sv[:, 2:4, 1:257], sv[:, 2:4, 1:257],
                                        sv[:, 0:2, 1:257], op=ALU.max)

            def seam_fill_from_A(src_A):
                nc.sync.dma_start(out=SEAM[0:1, 1:257],
                                  in_=src_A[127:128, 1:257])
                nc.sync.dma_start(out=SEAM[0:1, 259:515],
                                  in_=src_A[0:1, 259:515])
                seam_hmax()

            def seam_fill(src_X):
                # alpha rows 128 (g 0, t 16) and 129 (g 1, t 16) in sigma-0
                nc.sync.dma_start(
                    out=SEAM[0:1, 1:257],
                    in_=bass.AP(tensor=src_X.tensor,
                                offset=3 * FSZ + 16 * FW + 1,
                                ap=[[FSZ, 1], [1, 256]]))
                nc.sync.dma_start(
                    out=SEAM[0:1, 259:515],
                    in_=bass.AP(tensor=src_X.tensor,
                                offset=19 * FSZ + 16 * FW + 1,
                                ap=[[FSZ, 1], [1, 256]]))
                seam_hmax()

            def xwin(xt, plo, t, dj):
                # [64, 2, 256] window: partitions plo..plo+64, blocks t,t+1
                return bass.AP(tensor=xt.tensor,
                               offset=plo * FSZ + t * FW + dj,
                               ap=[[FSZ, 64], [FW, 2], [1, 256]])

            def flush_half(step, t0, t1):
                # refresh sigma copies 1..3 (or write output) for X0 blocks
                # t in [t0, t1)
                nf = (t1 - t0) * FW
                if step + 1 == steps:
                    nc.sync.dma_start(
                        out=out_ext[:, t0 * FW:t0 * FW + nf],
                        in_=X[0][:, t0 * FW:t0 * FW + nf])
                    return
                for c in range(1, NSIG):
                    npart = 128 - 16 * c
                    nc.gpsimd.dma_start(
                        out=bass.AP(tensor=X[c].tensor,
                                    offset=16 * c * FSZ + t0 * FW,
                                    ap=[[FSZ, npart], [1, nf]]),
                        in_=bass.AP(tensor=X[0].tensor,
                                    offset=t0 * FW,
                                    ap=[[FSZ, npart], [1, nf]]))
                    nc.sync.dma_start(
                        out=bass.AP(tensor=X[c].tensor,
                                    offset=(t0 + 1) * FW,
                                    ap=[[FSZ, 16 * c], [1, nf]]),
                        in_=bass.AP(tensor=X[0].tensor,
                                    offset=npart * FSZ + t0 * FW,
                                    ap=[[FSZ, 16 * c], [1, nf]]))

            # Deferred life/flush work: closures drained a few per sweep
            # window so life-broadcast matmuls interleave with dense L1/L2
            # work (no PE-queue stalls on the single pl PSUM slot) and the
            # sigma-copy refresh DMAs overlap compute. Half-1 work of step s
            # drains during the early windows of step s+1.
            pending = []   # items: (min_window_idx, closure)

            def drain(k, wi):
                n = 0
                while pending and n < k and pending[0][0] <= wi:
                    pending.pop(0)[1]()
                    n += 1

            for step in range(steps):
                # --- pre pool ---
                if step == 0:
                    extract_alpha(A_pre, X[0])
                seam_fill_from_A(A_pre)
                pool_half(VMpre, A_pre, 0, HM, HMu, HMd)
                pool_half(VMpre, A_pre, 1, HM, HMu, HMd)

                d_tiles = {}
                d_count = {}
                d_expect = {tb: 8 for tb in range(NBOUT)}
                d_expect[0] = 7
                d_expect[32] = 1

                def life_block(tb, step=step):
                    lo = tb * FW + 1
                    plan = life_plan[tb]
                    plt = ph_pool.tile([128, 2, 256], F32,
                                       name=f"pl_{step}_{tb}", tag="phA")
                    pl = plt[:, 0, :]
                    for i, (half, ridx) in enumerate(plan):
                        nc.tensor.matmul(
                            pl, rt[ridx],
                            LifeF[:, half * 256:half * 256 + 256],
                            start=(i == 0), stop=(i == len(plan) - 1))
                    nc.vector.tensor_tensor(X[0][:, lo:lo + 256],
                                            XN[:, lo:lo + 256], pl,
                                            op=ALU.mult)

                def post_half(half, step=step):
                    extract_alpha(A_post, XN, halves=(half,))
                    pool_half(VMpost, A_post, half, HMp, HMpu, HMpd)
                    qlo = 256 * half
                    qs = slice(qlo, qlo + 256)
                    nc.vector.tensor_tensor(LifeF[:, qs], VMpre[:, qs],
                                            VMpost[:, qs], op=ALU.min)
                    nc.vector.tensor_scalar(LifeF[:, qs], LifeF[:, qs],
                                            0.1, None, op0=ALU.is_gt)
                    if step + 1 < steps:
                        flo = 258 * half + 1
                        nc.vector.tensor_tensor(A_pre[:, flo:flo + 256],
                                                A_post[:, flo:flo + 256],
                                                LifeF[:, qs], op=ALU.mult)
                    if half == 0:
                        # post_half(0) runs during window idx 8; give the
                        # alpha->pool->LifeF chain ~2 windows before the first
                        # life matmul enters the in-order PE queue.
                        for tb in range(0, 13):
                            pending.append((10, lambda tb=tb: life_block(tb, step)))
                        pending.append((10, lambda: (life_block(13, step),
                                                     flush_half(step, 0, 13))))
                        for tb in (14, 15):
                            pending.append((10, lambda tb=tb: life_block(tb, step)))
                    else:
                        # drained at the start of the NEXT step; delay past
                        # window 2 so the post pool chain finishes first.
                        for tb in range(16, 21):
                            pending.append((2, lambda tb=tb: life_block(tb, step)))
                        pending.append((2, lambda: flush_half(step, 13, 21)))
                        for tb in range(21, 27):
                            pending.append((2, lambda tb=tb: life_block(tb, step)))
                        pending.append((2, lambda: flush_half(step, 21, 27)))
                        for tb in range(27, NBOUT):
                            pending.append((2, lambda tb=tb: life_block(tb, step)))
                        pending.append((2, lambda: flush_half(step, 27, NB - 1)))

                def l2(rho, ht, hslice):
                    tb, g = rho // 8, rho % 8
                    if tb not in d_tiles:
                        d_tiles[tb] = pd_pool.tile([128, 256], F32,
                                                   name=f"pd_s{step}_{tb}",
                                                   tag="pd")
                        d_count[tb] = 0
                    first = d_count[tb] == 0
                    d_count[tb] += 1
                    last = d_count[tb] == d_expect[tb]
                    nc.tensor.matmul(d_tiles[tb][:], w1t[g][:],
                                     ht[:, hslice], start=first, stop=last)
                    if last:
                        lo = tb * FW + 1
                        nc.vector.tensor_tensor(
                            XN[:, lo:lo + 256], d_tiles[tb][:],
                            X[0][:, lo:lo + 256], op=ALU.add)
                        if tb == 16:
                            seam_fill(XN)
                            post_half(0, step)

                # --- main sweep ---
                for M in range(0, NBOUT + 1, 2):
                    drain(3, M // 2)
                    for c in range(NSIG):
                        rA = 8 * M + 1 - c
                        rB = 8 * M + 5 - c
                        ph_A = ph_pool.tile([128, 2, 256], F32, tag="phA")
                        ph_B = ph_pool.tile([128, 2, 256], F32, tag="phB")
                        for dj in range(3):
                            nc.tensor.matmul(
                                ph_A[:], LW1[0:64, 128 * dj:128 * dj + 128],
                                xwin(X[c], 0, M, dj),
                                start=(dj == 0), stop=(dj == 2))
                            nc.tensor.matmul(
                                ph_B[:], LW1[64:128, 128 * dj:128 * dj + 128],
                                xwin(X[c], 64, M, dj),
                                start=(dj == 0), stop=(dj == 2))
                        ht_A = hpool.tile([128, 512], BF16, tag="htA")
                        relu_tile(ht_A[:], ph_A.rearrange("p a b -> p (a b)"))
                        ht_B = hpool.tile([128, 512], BF16, tag="htB")
                        relu_tile(ht_B[:], ph_B.rearrange("p a b -> p (a b)"))
                        for r0, ht in ((rA, ht_A), (rB, ht_B)):
                            if 1 <= r0 <= 256:
                                l2(r0, ht, slice(0, 256))
                            if 1 <= r0 + 8 <= 256:
                                l2(r0 + 8, ht, slice(256, 512))

                post_half(1, step)
                if step + 1 == steps:
                    drain(len(pending), 10 ** 9)

    nc.compile()
    return nc


_PROGRAM_CACHE = {}


def kernel(x, W0, b0, W1, steps, _trace=False):
    import concourse.bass_utils as bass_utils
    steps = int(steps)
    x = np.asarray(x, dtype=np.float32)
    W0 = np.asarray(W0, dtype=np.float32)
    b0 = np.asarray(b0, dtype=np.float32)
    W1 = np.asarray(W1, dtype=np.float32)
    B = x.shape[0]
    assert x.shape == (8, H, W, C), x.shape

    wts = build_weights(W0, b0, W1)
    key = steps
    if key not in _PROGRAM_CACHE:
        _PROGRAM_CACHE[key] = build_program(steps, wts["life_plan"],
                                            wts["r_stack"].shape[0])
    nc = _PROGRAM_CACHE[key]

    in_maps = []
    for b in range(B):
        xcs = marshal_x(x[b])
        m = {f"xc{c}": xcs[c] for c in range(NSIG)}
        m.update({
            "l1w": wts["l1w"],
            "w1w": wts["w1_stack"],
            "rw": wts["r_stack"],
            "b0w": wts["b0"],
        })
        in_maps.append(m)
    res = bass_utils.run_bass_kernel_spmd(nc, in_maps, list(range(8)),
                                          trace=_trace)
    kernel.last_result = res
    out = np.stack([unmarshal_x(res.results[b]["out"]) for b in range(B)])
    return out.astype(np.float32)


# revision 10
# speedup vs baseline: 1.5224x; 1.1402x over previous
"""Trainium2 Bass kernel for nn_CA_Model (neural cellular automaton).

Data-parallel over batch (8 images -> 8 cores). Per core the [256,256,16]
image lives in SBUF as FOUR fp16 row-shifted copies
  Xc[16*((r+c)%8)+ch, ((r+c)//8)*258 + 1 + w],  c in 0..3
so that every output row pair (rho, rho+4) finds its 3-row conv support on
partition strips {0,1} (group 1) and {2,3} (group 5) of the SAME copy:
the three dj matmuls of the 3x3-perceive + layer-1 fuse run as K=64
tile_position pairs that execute concurrently on disjoint PE row strips.

Layer 2 accumulates eight per-row matmuls (zero-padded W1^T columns) into a
PSUM tile per row-block. Alive maxpools run on a q2-interleaved alpha tile;
vertical max via DMA partition-shifted copies. State updates write the sigma=0
copy; copies 1..3 are refreshed by two batched partition-rotating DMAs per
copy per image half, overlapped with compute of the other half.
"""
import sys
for _p in ("/opt/trn_rl_repo", "/root/.axon_site/_ro/trn_rl_repo"):
    if _p not in sys.path:
        sys.path.append(_p)

import numpy as np

C = 16
HID = 128
H = W = 256
NB = 34            # row blocks in layout (34*8 = 272 slots; rows 0..257+shift used)
NBOUT = 33         # blocks that can hold image rows in sigma-0 layout
FW = 258           # padded row width in free dim
FSZ = NB * FW      # 8772 free elements per partition
NSIG = 4           # shifted copies


def _sobel():
    dx = np.outer([1, 2, 1], [-1, 0, 1]) / 8.0
    f1 = dx.T.astype(np.float32)
    f2 = dx.astype(np.float32)
    return f1, f2


def build_weights(W0, b0, W1):
    import ml_dtypes
    F1, F2 = _sobel()
    W0x, W0y1, W0y2 = W0[:, 0:16], W0[:, 16:32], W0[:, 32:48]
    # A[di][dj]: [HID, C] applied to x[row-1+di, w-1+dj]
    A = [[(np.float32(di == 1 and dj == 1) * W0x
           + F1[di, dj] * W0y1 + F2[di, dj] * W0y2).astype(np.float32)
          for dj in range(3)] for di in range(3)]

    # L1W [128, 3*128]: column block dj holds the K=64 lhsT for the A side
    # (support groups 0,1,2 -> partitions 16*di+c) and, shifted down 64, the
    # B side (groups 4,5,6 -> partitions 64+16*di+c).
    L1W = np.zeros((128, 3 * 128), np.float32)
    for dj in range(3):
        for di in range(3):
            L1W[16 * di:16 * di + 16, 128 * dj:128 * dj + 128] = A[di][dj].T
            L1W[64 + 16 * di:64 + 16 * di + 16, 128 * dj:128 * dj + 128] = A[di][dj].T

    # layer-2 lhsT: for a row with group g, W1pad[g][n, 16g+c] = W1[c, n]
    w1p = []
    for g in range(8):
        Wp = np.zeros((128, 128), np.float32)
        Wp[:, 16 * g:16 * g + 16] = W1.T
        w1p.append(Wp)

    # life-broadcast lhsT variants (q2 layout of LifeF -> X layout), per tb
    life_plan = []
    r_mats = []
    for tb in range(NBOUT):
        plan = []
        buckets = {}
        for g in range(8):
            rho = 8 * tb + g
            if rho < 1 or rho > 256:
                continue
            q = rho - 1
            half, qh = q // 128, q % 128
            buckets.setdefault(half, []).append((g, qh))
        for half, rows in sorted(buckets.items()):
            Rm = np.zeros((128, 128), np.float32)
            for g, qh in rows:
                q2 = (qh % 8) * 16 + qh // 8
                Rm[q2, 16 * g:16 * g + 16] = 1.0
            plan.append((half, len(r_mats)))
            r_mats.append(Rm)
        life_plan.append(plan)

    f16 = np.float16
    return dict(l1w=L1W.astype(f16),
                w1_stack=np.stack(w1p).astype(__import__('ml_dtypes').bfloat16),
                r_stack=np.stack(r_mats).astype(f16),
                life_plan=life_plan,
                b0=b0.reshape(128, 1).astype(np.float32))


def marshal_x(img):
    """[256,256,16] image -> 4 shifted copies [128, FSZ] fp16."""
    out = []
    for c in range(NSIG):
        xp = np.zeros((NB * 8, FW, C), np.float16)
        xp[1 + c:257 + c, 1:257, :] = img
        xc = xp.reshape(NB, 8, FW, C).transpose(1, 3, 0, 2).reshape(128, FSZ)
        out.append(np.ascontiguousarray(xc))
    return out


def unmarshal_x(xc):
    """X0 [128, FSZ] -> [256,256,16] image."""
    xp = xc.reshape(8, C, NB, FW).transpose(2, 0, 3, 1)
    xp = xp.reshape(NB * 8, FW, C)
    return np.ascontiguousarray(xp[1:257, 1:257, :])


def build_program(steps, life_plan, n_r,
                  relu_split=(6, 2, 0), debug_phases=99):
    """relu_split: of every sum(relu_split) relu tiles, how many go to
    (scalar, vector, gpsimd)."""
    import concourse.bass as bass
    import concourse.bacc as bacc
    import concourse.tile as tile
    from concourse import mybir
    F32 = mybir.dt.float32
    F16 = mybir.dt.float16
    BF16 = mybir.dt.bfloat16
    AF = mybir.ActivationFunctionType
    ALU = mybir.AluOpType
    nc = bacc.Bacc(None, target_bir_lowering=False, debug=False, num_devices=8,
                   num_swdge_queues=4)

    x_ext = [nc.declare_dram_parameter(f"xc{c}", [128, FSZ], F16, isOutput=False)
             for c in range(NSIG)]
    l1_ext = nc.declare_dram_parameter("l1w", [128, 3 * 128], F16, isOutput=False)
    w1_ext = nc.declare_dram_parameter("w1w", [8, 128, 128], BF16, isOutput=False)
    r_ext = nc.declare_dram_parameter("rw", [n_r, 128, 128], F16, isOutput=False)
    b0_ext = nc.declare_dram_parameter("b0w", [128, 1], F32, isOutput=False)
    out_ext = nc.declare_dram_parameter("out", [128, FSZ], F16, isOutput=True)

    with tile.TileContext(nc) as tc:
        with tc.tile_pool(name="hpool", bufs=6) as hpool, \
             tc.tile_pool(name="ph_pool", bufs=2, space="PSUM") as ph_pool, \
             tc.tile_pool(name="pd_pool", bufs=3, space="PSUM") as pd_pool:

            # --- persistent state ---
            X = [nc.alloc_sbuf_tensor(f"X{c}", [128, FSZ], F16).ap()
                 for c in range(NSIG)]
            XN = nc.alloc_sbuf_tensor("XN", [128, FSZ], F16).ap()

            LW1 = nc.alloc_sbuf_tensor("LW1", [128, 3 * 128], F16).ap()
            LW2 = nc.alloc_sbuf_tensor("LW2", [128, 8 * 128], BF16).ap()
            LWR = nc.alloc_sbuf_tensor("LWR", [128, n_r * 128], F16).ap()
            w1t = [LW2[:, 128 * g:128 * g + 128] for g in range(8)]
            rt = [LWR[:, 128 * i:128 * i + 128] for i in range(n_r)]
            b0t = nc.alloc_sbuf_tensor("b0t", [128, 1], F32).ap()

            A_pre = nc.alloc_sbuf_tensor("A_pre", [128, 516], F16).ap()
            A_post = nc.alloc_sbuf_tensor("A_post", [128, 516], F16).ap()
            HM = nc.alloc_sbuf_tensor("HM", [128, 512], F16).ap()
            HMu = nc.alloc_sbuf_tensor("HMu", [128, 512], F16).ap()
            HMd = nc.alloc_sbuf_tensor("HMd", [128, 512], F16).ap()
            HMp = nc.alloc_sbuf_tensor("HMp", [128, 512], F16).ap()
            HMpu = nc.alloc_sbuf_tensor("HMpu", [128, 512], F16).ap()
            HMpd = nc.alloc_sbuf_tensor("HMpd", [128, 512], F16).ap()
            # seam scratch: partition 0: [0:258] a128 | [258:516] a129 |
            # [516:774] hm128 | [774:1032] hm129
            SEAM = nc.alloc_sbuf_tensor("SEAM", [128, 1032], F16).ap()
            VMpre = nc.alloc_sbuf_tensor("VMpre", [128, 512], F16).ap()
            VMpost = nc.alloc_sbuf_tensor("VMpost", [128, 512], F16).ap()
            LifeF = nc.alloc_sbuf_tensor("LifeF", [128, 512], F16).ap()
            Zrow = nc.alloc_sbuf_tensor("Zrow", [128, 516], F16).ap()

            # --- loads / init ---
            for c in range(NSIG):
                nc.gpsimd.dma_start(out=X[c][:], in_=x_ext[c][:])
            nc.gpsimd.dma_start(out=LW1[:], in_=l1_ext[:])
            nc.gpsimd.dma_start(out=LW2[:], in_=bass.AP(
                tensor=w1_ext, offset=0,
                ap=[[128, 128], [128 * 128, 8], [1, 128]]))
            nc.gpsimd.dma_start(out=LWR[:], in_=bass.AP(
                tensor=r_ext, offset=0,
                ap=[[128, 128], [128 * 128, n_r], [1, 128]]))
            nc.gpsimd.dma_start(out=b0t[:], in_=b0_ext[:])
            nc.vector.memset(Zrow[:], 0.0)
            nc.vector.memset(SEAM[0:32, :], 0.0)
            nc.vector.memset(A_post[:], 0.0)
            nc.vector.memset(A_pre[:], 0.0)

            relu_ctr = [0]
            rs_total = sum(relu_split)
            rs_s, rs_v = relu_split[0], relu_split[0] + relu_split[1]

            def relu_tile(dst, src):
                k = relu_ctr[0] % rs_total
                relu_ctr[0] += 1
                if k < rs_s:
                    nc.scalar.activation(dst, src, AF.Relu, bias=b0t[:], scale=1.0)
                elif k < rs_v:
                    nc.vector.tensor_scalar(dst, src, b0t[:], 0.0,
                                            op0=ALU.add, op1=ALU.max)
                else:
                    nc.vector.tensor_scalar(dst, src, b0t[:], 0.0,
                                            op0=ALU.add, op1=ALU.max)

            def extract_alpha(dst_A, src_X, halves=(0, 1)):
                # q2-layout: dst_A[gp*16 + j, half*258 + 1 + w] holds alpha of
                # row rho = 128*half + 8j + gp + 1 (alpha: channel 3).
                for half in halves:
                    for gp in range(8):
                        g = (gp + 1) % 8
                        t0 = 16 * half + (1 if gp == 7 else 0)
                        dst = bass.AP(
                            tensor=dst_A.tensor,
                            offset=16 * gp * 516 + 258 * half + 1,
                            ap=[[516, 16], [1, 256]])
                        srcap = bass.AP(
                            tensor=src_X.tensor,
                            offset=(16 * g + 3) * FSZ + t0 * FW + 1,
                            ap=[[FSZ, 1], [FW, 16], [1, 256]])
                        eng = nc.sync if gp % 2 == 0 else nc.gpsimd
                        eng.dma_start(out=dst, in_=srcap)

            def pool_half(dst_VM, src_A, half, hm, hmu, hmd):
                lo, hi = 258 * half, 258 * half + 258
                qlo, qhi = 256 * half, 256 * half + 256
                av = src_A[:, lo:hi]
                nc.vector.tensor_tensor(hm[:, qlo:qhi], av[:, 0:256],
                                        av[:, 2:258], op=ALU.max)
                nc.vector.tensor_tensor(hm[:, qlo:qhi], hm[:, qlo:qhi],
                                        av[:, 1:257], op=ALU.max)
                nc.gpsimd.dma_start(out=hmu[0:112, qlo:qhi], in_=hm[16:128, qlo:qhi])
                nc.sync.dma_start(out=hmu[112:127, qlo:qhi], in_=hm[1:16, qlo:qhi])
                nc.gpsimd.dma_start(out=hmd[16:128, qlo:qhi], in_=hm[0:112, qlo:qhi])
                nc.sync.dma_start(out=hmd[1:16, qlo:qhi], in_=hm[112:127, qlo:qhi])
                if half == 0:
                    nc.gpsimd.dma_start(out=hmu[127:128, 0:256],
                                        in_=SEAM[0:1, 775:1031])
                    nc.sync.dma_start(out=hmd[0:1, 0:256], in_=Zrow[0:1, 0:256])
                else:
                    nc.gpsimd.dma_start(out=hmd[0:1, 256:512],
                                        in_=SEAM[0:1, 517:773])
                    nc.sync.dma_start(out=hmu[127:128, 256:512],
                                      in_=Zrow[0:1, 0:256])
                nc.vector.tensor_tensor(dst_VM[:, qlo:qhi], hm[:, qlo:qhi],
                                        hmu[:, qlo:qhi], op=ALU.max)
                nc.vector.tensor_tensor(dst_VM[:, qlo:qhi], dst_VM[:, qlo:qhi],
                                        hmd[:, qlo:qhi], op=ALU.max)

            def seam_hmax():
                sv = SEAM[0:1, :].rearrange("p (a w) -> p a w", a=4)
                nc.vector.tensor_tensor(sv[:, 2:4, 1:257], sv[:, 0:2, 0:256],
                                        sv[:, 0:2, 2:258], op=ALU.max)
                nc.vector.tensor_tensor(sv[:, 2:4, 1:257], sv[:, 2:4, 1:257],
                                        sv[:, 0:2, 1:257], op=ALU.max)

            def seam_fill_from_A(src_A):
                nc.sync.dma_start(out=SEAM[0:1, 1:257],
                                  in_=src_A[127:128, 1:257])
                nc.sync.dma_start(out=SEAM[0:1, 259:515],
                                  in_=src_A[0:1, 259:515])
                seam_hmax()

            def seam_fill(src_X):
                # alpha rows 128 (g 0, t 16) and 129 (g 1, t 16) in sigma-0
                nc.sync.dma_start(
                    out=SEAM[0:1, 1:257],
                    in_=bass.AP(tensor=src_X.tensor,
                                offset=3 * FSZ + 16 * FW + 1,
                                ap=[[FSZ, 1], [1, 256]]))
                nc.sync.dma_start(
                    out=SEAM[0:1, 259:515],
                    in_=bass.AP(tensor=src_X.tensor,
                                offset=19 * FSZ + 16 * FW + 1,
                                ap=[[FSZ, 1], [1, 256]]))
                seam_hmax()

            def xwin(xt, plo, t, dj):
                # [64, 2, 256] window: partitions plo..plo+64, blocks t,t+1
                return bass.AP(tensor=xt.tensor,
                               offset=plo * FSZ + t * FW + dj,
                               ap=[[FSZ, 64], [FW, 2], [1, 256]])

            def flush_half(step, t0, t1):
                # refresh sigma copies 1..3 (or write output) for X0 blocks
                # t in [t0, t1)
                nf = (t1 - t0) * FW
                if step + 1 == steps:
                    nc.sync.dma_start(
                        out=out_ext[:, t0 * FW:t0 * FW + nf],
                        in_=X[0][:, t0 * FW:t0 * FW + nf])
                    return
                for c in range(1, NSIG):
                    npart = 128 - 16 * c
                    nc.gpsimd.dma_start(
                        out=bass.AP(tensor=X[c].tensor,
                                    offset=16 * c * FSZ + t0 * FW,
                                    ap=[[FSZ, npart], [1, nf]]),
                        in_=bass.AP(tensor=X[0].tensor,
                                    offset=t0 * FW,
                                    ap=[[FSZ, npart], [1, nf]]))
                    nc.sync.dma_start(
                        out=bass.AP(tensor=X[c].tensor,
                                    offset=(t0 + 1) * FW,
                                    ap=[[FSZ, 16 * c], [1, nf]]),
                        in_=bass.AP(tensor=X[0].tensor,
                                    offset=npart * FSZ + t0 * FW,
                                    ap=[[FSZ, 16 * c], [1, nf]]))

            # Deferred life/flush work: closures drained a few per sweep
            # window so life-broadcast matmuls interleave with dense L1/L2
            # work (no PE-queue stalls on the single pl PSUM slot) and the
            # sigma-copy refresh DMAs overlap compute. Half-1 work of step s
            # drains during the early windows of step s+1.
            pending = []   # items: (min_window_idx, closure)

            def drain(k, wi):
                n = 0
                while pending and n < k and pending[0][0] <= wi:
                    pending.pop(0)[1]()
                    n += 1

            for step in range(steps):
                # --- pre pool ---
                if step == 0:
                    extract_alpha(A_pre, X[0])
                seam_fill_from_A(A_pre)
                pool_half(VMpre, A_pre, 0, HM, HMu, HMd)
                pool_half(VMpre, A_pre, 1, HM, HMu, HMd)

                d_tiles = {}
                d_count = {}
                d_expect = {tb: 8 for tb in range(NBOUT)}
                d_expect[0] = 7
                d_expect[32] = 1

                def life_block(tb, step=step, ntb=2):
                    # one pl tile + one DVE mult covers tbs tb .. tb+ntb-1
                    ntb = min(ntb, NBOUT - tb)
                    plt = ph_pool.tile([128, 2, 256], F32,
                                       name=f"pl_{step}_{tb}", tag="phA")
                    for k in range(ntb):
                        plan = life_plan[tb + k]
                        for i, (half, ridx) in enumerate(plan):
                            nc.tensor.matmul(
                                plt[:, k, :], rt[ridx],
                                LifeF[:, half * 256:half * 256 + 256],
                                start=(i == 0), stop=(i == len(plan) - 1))
                    def xv(t):
                        return bass.AP(tensor=t.tensor,
                                       offset=tb * FW + 1,
                                       ap=[[FSZ, 128], [FW, ntb], [1, 256]])
                    nc.vector.tensor_tensor(xv(X[0]), xv(XN),
                                            plt[:, 0:ntb, :], op=ALU.mult)

                def post_half(half, step=step):
                    extract_alpha(A_post, XN, halves=(half,))
                    pool_half(VMpost, A_post, half, HMp, HMpu, HMpd)
                    qlo = 256 * half
                    qs = slice(qlo, qlo + 256)
                    nc.vector.tensor_tensor(LifeF[:, qs], VMpre[:, qs],
                                            VMpost[:, qs], op=ALU.min)
                    nc.vector.tensor_scalar(LifeF[:, qs], LifeF[:, qs],
                                            0.1, None, op0=ALU.is_gt)
                    if step + 1 < steps:
                        flo = 258 * half + 1
                        nc.vector.tensor_tensor(A_pre[:, flo:flo + 256],
                                                A_post[:, flo:flo + 256],
                                                LifeF[:, qs], op=ALU.mult)
                    if half == 0:
                        # post_half(0) runs during window idx 8; give the
                        # alpha->pool->LifeF chain ~2 windows before the first
                        # life matmul enters the in-order PE queue.
                        for tb in range(0, 12, 2):
                            pending.append((10, lambda tb=tb: life_block(tb, step)))
                        pending.append((10, lambda: (life_block(12, step),
                                                     flush_half(step, 0, 13))))
                        pending.append((10, lambda: life_block(14, step)))
                    else:
                        # drained at the start of the NEXT step; delay past
                        # window 2 so the post pool chain finishes first.
                        for tb in range(16, 22, 2):
                            pending.append((2, lambda tb=tb: life_block(tb, step)))
                        pending.append((2, lambda: flush_half(step, 13, 21)))
                        for tb in range(22, 28, 2):
                            pending.append((2, lambda tb=tb: life_block(tb, step)))
                        pending.append((2, lambda: flush_half(step, 21, 27)))
                        for tb in range(28, NBOUT, 2):
                            pending.append((2, lambda tb=tb: life_block(tb, step)))
                        pending.append((2, lambda: flush_half(step, 27, NB - 1)))

                def l2(rho, ht, hslice):
                    tb, g = rho // 8, rho % 8
                    if tb not in d_tiles:
                        d_tiles[tb] = pd_pool.tile([128, 256], F32,
                                                   name=f"pd_s{step}_{tb}",
                                                   tag="pd")
                        d_count[tb] = 0
                    first = d_count[tb] == 0
                    d_count[tb] += 1
                    last = d_count[tb] == d_expect[tb]
                    nc.tensor.matmul(d_tiles[tb][:], w1t[g][:],
                                     ht[:, hslice], start=first, stop=last)
                    if last:
                        lo = tb * FW + 1
                        nc.vector.tensor_tensor(
                            XN[:, lo:lo + 256], d_tiles[tb][:],
                            X[0][:, lo:lo + 256], op=ALU.add)
                        if tb == 16:
                            seam_fill(XN)
                            post_half(0, step)

                # --- main sweep (L2 deferred one c-iter behind L1, so the
                # L2 matmuls never wait on relu at the head of the PE queue)
                held = []
                for M in range(0, NBOUT + 1, 2):
                    drain(3, M // 2)
                    for c in range(NSIG):
                        rA = 8 * M + 1 - c
                        rB = 8 * M + 5 - c
                        ph_A = ph_pool.tile([128, 2, 256], F32, tag="phA")
                        ph_B = ph_pool.tile([128, 2, 256], F32, tag="phB")
                        for dj in range(3):
                            nc.tensor.matmul(
                                ph_A[:], LW1[0:64, 128 * dj:128 * dj + 128],
                                xwin(X[c], 0, M, dj),
                                start=(dj == 0), stop=(dj == 2))
                            nc.tensor.matmul(
                                ph_B[:], LW1[64:128, 128 * dj:128 * dj + 128],
                                xwin(X[c], 64, M, dj),
                                start=(dj == 0), stop=(dj == 2))
                        ht_A = hpool.tile([128, 512], BF16, tag="htA")
                        relu_tile(ht_A[:], ph_A.rearrange("p a b -> p (a b)"))
                        ht_B = hpool.tile([128, 512], BF16, tag="htB")
                        relu_tile(ht_B[:], ph_B.rearrange("p a b -> p (a b)"))
                        for args in held:
                            l2(*args)
                        held = []
                        for r0, ht in ((rA, ht_A), (rB, ht_B)):
                            if 1 <= r0 <= 256:
                                held.append((r0, ht, slice(0, 256)))
                            if 1 <= r0 + 8 <= 256:
                                held.append((r0 + 8, ht, slice(256, 512)))

                for args in held:
                    l2(*args)
                held = []
                post_half(1, step)
                if step + 1 == steps:
                    drain(len(pending), 10 ** 9)

    nc.compile()
    return nc


_PROGRAM_CACHE = {}


def kernel(x, W0, b0, W1, steps, _trace=False):
    import concourse.bass_utils as bass_utils
    steps = int(steps)
    x = np.asarray(x, dtype=np.float32)
    W0 = np.asarray(W0, dtype=np.float32)
    b0 = np.asarray(b0, dtype=np.float32)
    W1 = np.asarray(W1, dtype=np.float32)
    B = x.shape[0]
    assert x.shape == (8, H, W, C), x.shape

    wts = build_weights(W0, b0, W1)
    key = steps
    if key not in _PROGRAM_CACHE:
        _PROGRAM_CACHE[key] = build_program(steps, wts["life_plan"],
                                            wts["r_stack"].shape[0])
    nc = _PROGRAM_CACHE[key]

    in_maps = []
    for b in range(B):
        xcs = marshal_x(x[b])
        m = {f"xc{c}": xcs[c] for c in range(NSIG)}
        m.update({
            "l1w": wts["l1w"],
            "w1w": wts["w1_stack"],
            "rw": wts["r_stack"],
            "b0w": wts["b0"],
        })
        in_maps.append(m)
    res = bass_utils.run_bass_kernel_spmd(nc, in_maps, list(range(8)),
                                          trace=_trace)
    kernel.last_result = res
    out = np.stack([unmarshal_x(res.results[b]["out"]) for b in range(B)])
    return out.astype(np.float32)


# revision 11
# speedup vs baseline: 1.6129x; 1.0594x over previous
"""Trainium2 Bass kernel for nn_CA_Model (neural cellular automaton).

Data-parallel over batch (8 images -> 8 cores). Per core the [256,256,16]
image lives in SBUF as FOUR fp16 row-shifted copies
  Xc[16*((r+c)%8)+ch, ((r+c)//8)*258 + 1 + w],  c in 0..3
so that every output row pair (rho, rho+4) finds its 3-row conv support on
partition strips {0,1} (group 1) and {2,3} (group 5) of the SAME copy:
the three dj matmuls of the 3x3-perceive + layer-1 fuse run as K=64
tile_position pairs that execute concurrently on disjoint PE row strips.

Layer 2 accumulates eight per-row matmuls (zero-padded W1^T columns) into a
PSUM tile per row-block. Alive maxpools run on a q2-interleaved alpha tile;
vertical max via DMA partition-shifted copies. State updates write the sigma=0
copy; copies 1..3 are refreshed by two batched partition-rotating DMAs per
copy per image half, overlapped with compute of the other half.
"""
import sys
for _p in ("/opt/trn_rl_repo", "/root/.axon_site/_ro/trn_rl_repo"):
    if _p not in sys.path:
        sys.path.append(_p)

import numpy as np

C = 16
HID = 128
H = W = 256
NB = 34            # row blocks in layout (34*8 = 272 slots; rows 0..257+shift used)
NBOUT = 33         # blocks that can hold image rows in sigma-0 layout
FW = 258           # padded row width in free dim
FSZ = NB * FW      # 8772 free elements per partition
NSIG = 4           # shifted copies


def _sobel():
    dx = np.outer([1, 2, 1], [-1, 0, 1]) / 8.0
    f1 = dx.T.astype(np.float32)
    f2 = dx.astype(np.float32)
    return f1, f2


def build_weights(W0, b0, W1):
    import ml_dtypes
    F1, F2 = _sobel()
    W0x, W0y1, W0y2 = W0[:, 0:16], W0[:, 16:32], W0[:, 32:48]
    # A[di][dj]: [HID, C] applied to x[row-1+di, w-1+dj]
    A = [[(np.float32(di == 1 and dj == 1) * W0x
           + F1[di, dj] * W0y1 + F2[di, dj] * W0y2).astype(np.float32)
          for dj in range(3)] for di in range(3)]

    # L1W [128, 3*128]: column block dj holds the K=64 lhsT for the A side
    # (support groups 0,1,2 -> partitions 16*di+c) and, shifted down 64, the
    # B side (groups 4,5,6 -> partitions 64+16*di+c).
    L1W = np.zeros((128, 3 * 128), np.float32)
    for dj in range(3):
        for di in range(3):
            L1W[16 * di:16 * di + 16, 128 * dj:128 * dj + 128] = A[di][dj].T
            L1W[64 + 16 * di:64 + 16 * di + 16, 128 * dj:128 * dj + 128] = A[di][dj].T

    # layer-2 lhsT: for a row with group g, W1pad[g][n, 16g+c] = W1[c, n]
    w1p = []
    for g in range(8):
        Wp = np.zeros((128, 128), np.float32)
        Wp[:, 16 * g:16 * g + 16] = W1.T
        w1p.append(Wp)

    # life-broadcast lhsT variants (q2 layout of LifeF -> X layout), per tb
    life_plan = []
    r_mats = []
    for tb in range(NBOUT):
        plan = []
        buckets = {}
        for g in range(8):
            rho = 8 * tb + g
            if rho < 1 or rho > 256:
                continue
            q = rho - 1
            half, qh = q // 128, q % 128
            buckets.setdefault(half, []).append((g, qh))
        for half, rows in sorted(buckets.items()):
            Rm = np.zeros((128, 128), np.float32)
            for g, qh in rows:
                q2 = (qh % 8) * 16 + qh // 8
                Rm[q2, 16 * g:16 * g + 16] = 1.0
            plan.append((half, len(r_mats)))
            r_mats.append(Rm)
        life_plan.append(plan)

    f16 = np.float16
    return dict(l1w=L1W.astype(f16),
                w1_stack=np.stack(w1p).astype(__import__('ml_dtypes').bfloat16),
                r_stack=np.stack(r_mats).astype(f16),
                life_plan=life_plan,
                b0=b0.reshape(128, 1).astype(np.float32))


def marshal_x(img):
    """[256,256,16] image -> 4 shifted copies [128, FSZ] fp16."""
    out = []
    for c in range(NSIG):
        xp = np.zeros((NB * 8, FW, C), np.float16)
        xp[1 + c:257 + c, 1:257, :] = img
        xc = xp.reshape(NB, 8, FW, C).transpose(1, 3, 0, 2).reshape(128, FSZ)
        out.append(np.ascontiguousarray(xc))
    return out


def unmarshal_x(xc):
    """X0 [128, FSZ] -> [256,256,16] image."""
    xp = xc.reshape(8, C, NB, FW).transpose(2, 0, 3, 1)
    xp = xp.reshape(NB * 8, FW, C)
    return np.ascontiguousarray(xp[1:257, 1:257, :])


def build_program(steps, life_plan, n_r,
                  relu_split=(7, 1, 0), debug_phases=99):
    """relu_split: of every sum(relu_split) relu tiles, how many go to
    (scalar, vector, gpsimd)."""
    import concourse.bass as bass
    import concourse.bacc as bacc
    import concourse.tile as tile
    from concourse import mybir
    F32 = mybir.dt.float32
    F16 = mybir.dt.float16
    BF16 = mybir.dt.bfloat16
    AF = mybir.ActivationFunctionType
    ALU = mybir.AluOpType
    nc = bacc.Bacc(None, target_bir_lowering=False, debug=False, num_devices=8,
                   num_swdge_queues=4)

    x_ext = [nc.declare_dram_parameter(f"xc{c}", [128, FSZ], F16, isOutput=False)
             for c in range(NSIG)]
    l1_ext = nc.declare_dram_parameter("l1w", [128, 3 * 128], F16, isOutput=False)
    w1_ext = nc.declare_dram_parameter("w1w", [8, 128, 128], BF16, isOutput=False)
    r_ext = nc.declare_dram_parameter("rw", [n_r, 128, 128], F16, isOutput=False)
    b0_ext = nc.declare_dram_parameter("b0w", [128, 1], F32, isOutput=False)
    out_ext = nc.declare_dram_parameter("out", [128, FSZ], F16, isOutput=True)

    with tile.TileContext(nc) as tc:
        with tc.tile_pool(name="hpool", bufs=6) as hpool, \
             tc.tile_pool(name="ph_pool", bufs=2, space="PSUM") as ph_pool, \
             tc.tile_pool(name="pd_pool", bufs=3, space="PSUM") as pd_pool:

            # --- persistent state ---
            X = [nc.alloc_sbuf_tensor(f"X{c}", [128, FSZ], F16).ap()
                 for c in range(NSIG)]
            XN = nc.alloc_sbuf_tensor("XN", [128, FSZ], F16).ap()

            LW1 = nc.alloc_sbuf_tensor("LW1", [128, 3 * 128], F16).ap()
            LW2 = nc.alloc_sbuf_tensor("LW2", [128, 8 * 128], BF16).ap()
            LWR = nc.alloc_sbuf_tensor("LWR", [128, n_r * 128], F16).ap()
            w1t = [LW2[:, 128 * g:128 * g + 128] for g in range(8)]
            rt = [LWR[:, 128 * i:128 * i + 128] for i in range(n_r)]
            b0t = nc.alloc_sbuf_tensor("b0t", [128, 1], F32).ap()

            A_pre = nc.alloc_sbuf_tensor("A_pre", [128, 516], F16).ap()
            A_post = nc.alloc_sbuf_tensor("A_post", [128, 516], F16).ap()
            HM = nc.alloc_sbuf_tensor("HM", [128, 512], F16).ap()
            HMu = nc.alloc_sbuf_tensor("HMu", [128, 512], F16).ap()
            HMd = nc.alloc_sbuf_tensor("HMd", [128, 512], F16).ap()
            HMp = nc.alloc_sbuf_tensor("HMp", [128, 512], F16).ap()
            HMpu = nc.alloc_sbuf_tensor("HMpu", [128, 512], F16).ap()
            HMpd = nc.alloc_sbuf_tensor("HMpd", [128, 512], F16).ap()
            # seam scratch: partition 0: [0:258] a128 | [258:516] a129 |
            # [516:774] hm128 | [774:1032] hm129
            SEAM = nc.alloc_sbuf_tensor("SEAM", [128, 1032], F16).ap()
            VMpre = nc.alloc_sbuf_tensor("VMpre", [128, 512], F16).ap()
            VMpost = nc.alloc_sbuf_tensor("VMpost", [128, 512], F16).ap()
            LifeF = nc.alloc_sbuf_tensor("LifeF", [128, 512], F16).ap()
            Zrow = nc.alloc_sbuf_tensor("Zrow", [128, 516], F16).ap()

            # --- loads / init ---
            for c in range(NSIG):
                nc.gpsimd.dma_start(out=X[c][:], in_=x_ext[c][:])
            nc.gpsimd.dma_start(out=LW1[:], in_=l1_ext[:])
            nc.gpsimd.dma_start(out=LW2[:], in_=bass.AP(
                tensor=w1_ext, offset=0,
                ap=[[128, 128], [128 * 128, 8], [1, 128]]))
            nc.gpsimd.dma_start(out=LWR[:], in_=bass.AP(
                tensor=r_ext, offset=0,
                ap=[[128, 128], [128 * 128, n_r], [1, 128]]))
            nc.gpsimd.dma_start(out=b0t[:], in_=b0_ext[:])
            nc.vector.memset(Zrow[:], 0.0)
            nc.vector.memset(SEAM[0:32, :], 0.0)
            nc.vector.memset(A_post[:], 0.0)
            nc.vector.memset(A_pre[:], 0.0)

            relu_ctr = [0]
            rs_total = sum(relu_split)
            rs_s, rs_v = relu_split[0], relu_split[0] + relu_split[1]

            def relu_tile(dst, src):
                k = relu_ctr[0] % rs_total
                relu_ctr[0] += 1
                if k < rs_s:
                    nc.scalar.activation(dst, src, AF.Relu, bias=b0t[:], scale=1.0)
                elif k < rs_v:
                    nc.vector.tensor_scalar(dst, src, b0t[:], 0.0,
                                            op0=ALU.add, op1=ALU.max)
                else:
                    nc.vector.tensor_scalar(dst, src, b0t[:], 0.0,
                                            op0=ALU.add, op1=ALU.max)

            def extract_alpha(dst_A, src_X, halves=(0, 1)):
                # q2-layout: dst_A[gp*16 + j, half*258 + 1 + w] holds alpha of
                # row rho = 128*half + 8j + gp + 1 (alpha: channel 3).
                for half in halves:
                    for gp in range(8):
                        g = (gp + 1) % 8
                        t0 = 16 * half + (1 if gp == 7 else 0)
                        dst = bass.AP(
                            tensor=dst_A.tensor,
                            offset=16 * gp * 516 + 258 * half + 1,
                            ap=[[516, 16], [1, 256]])
                        srcap = bass.AP(
                            tensor=src_X.tensor,
                            offset=(16 * g + 3) * FSZ + t0 * FW + 1,
                            ap=[[FSZ, 1], [FW, 16], [1, 256]])
                        eng = nc.sync if gp % 2 == 0 else nc.gpsimd
                        eng.dma_start(out=dst, in_=srcap)

            def pool_half(dst_VM, src_A, half, hm, hmu, hmd):
                lo, hi = 258 * half, 258 * half + 258
                qlo, qhi = 256 * half, 256 * half + 256
                av = src_A[:, lo:hi]
                nc.vector.tensor_tensor(hm[:, qlo:qhi], av[:, 0:256],
                                        av[:, 2:258], op=ALU.max)
                nc.vector.tensor_tensor(hm[:, qlo:qhi], hm[:, qlo:qhi],
                                        av[:, 1:257], op=ALU.max)
                nc.gpsimd.dma_start(out=hmu[0:112, qlo:qhi], in_=hm[16:128, qlo:qhi])
                nc.sync.dma_start(out=hmu[112:127, qlo:qhi], in_=hm[1:16, qlo:qhi])
                nc.gpsimd.dma_start(out=hmd[16:128, qlo:qhi], in_=hm[0:112, qlo:qhi])
                nc.sync.dma_start(out=hmd[1:16, qlo:qhi], in_=hm[112:127, qlo:qhi])
                if half == 0:
                    nc.gpsimd.dma_start(out=hmu[127:128, 0:256],
                                        in_=SEAM[0:1, 775:1031])
                    nc.sync.dma_start(out=hmd[0:1, 0:256], in_=Zrow[0:1, 0:256])
                else:
                    nc.gpsimd.dma_start(out=hmd[0:1, 256:512],
                                        in_=SEAM[0:1, 517:773])
                    nc.sync.dma_start(out=hmu[127:128, 256:512],
                                      in_=Zrow[0:1, 0:256])
                nc.vector.tensor_tensor(dst_VM[:, qlo:qhi], hm[:, qlo:qhi],
                                        hmu[:, qlo:qhi], op=ALU.max)
                nc.vector.tensor_tensor(dst_VM[:, qlo:qhi], dst_VM[:, qlo:qhi],
                                        hmd[:, qlo:qhi], op=ALU.max)

            def seam_hmax():
                sv = SEAM[0:1, :].rearrange("p (a w) -> p a w", a=4)
                nc.vector.tensor_tensor(sv[:, 2:4, 1:257], sv[:, 0:2, 0:256],
                                        sv[:, 0:2, 2:258], op=ALU.max)
                nc.vector.tensor_tensor(sv[:, 2:4, 1:257], sv[:, 2:4, 1:257],
                                        sv[:, 0:2, 1:257], op=ALU.max)

            def seam_fill_from_A(src_A):
                nc.sync.dma_start(out=SEAM[0:1, 1:257],
                                  in_=src_A[127:128, 1:257])
                nc.sync.dma_start(out=SEAM[0:1, 259:515],
                                  in_=src_A[0:1, 259:515])
                seam_hmax()

            def seam_fill(src_X):
                # alpha rows 128 (g 0, t 16) and 129 (g 1, t 16) in sigma-0
                nc.sync.dma_start(
                    out=SEAM[0:1, 1:257],
                    in_=bass.AP(tensor=src_X.tensor,
                                offset=3 * FSZ + 16 * FW + 1,
                                ap=[[FSZ, 1], [1, 256]]))
                nc.sync.dma_start(
                    out=SEAM[0:1, 259:515],
                    in_=bass.AP(tensor=src_X.tensor,
                                offset=19 * FSZ + 16 * FW + 1,
                                ap=[[FSZ, 1], [1, 256]]))
                seam_hmax()

            def xwin(xt, plo, t, dj):
                # [64, 2, 256] window: partitions plo..plo+64, blocks t,t+1
                return bass.AP(tensor=xt.tensor,
                               offset=plo * FSZ + t * FW + dj,
                               ap=[[FSZ, 64], [FW, 2], [1, 256]])

            def flush_half(step, t0, t1):
                # refresh sigma copies 1..3 (or write output) for X0 blocks
                # t in [t0, t1)
                nf = (t1 - t0) * FW
                if step + 1 == steps:
                    # scalar queue: keeps the big DRAM writes off the
                    # gpsimd/sync queues that carry the alpha/pool chain
                    nc.scalar.dma_start(
                        out=out_ext[:, t0 * FW:t0 * FW + nf],
                        in_=X[0][:, t0 * FW:t0 * FW + nf])
                    return
                for c in range(1, NSIG):
                    npart = 128 - 16 * c
                    nc.gpsimd.dma_start(
                        out=bass.AP(tensor=X[c].tensor,
                                    offset=16 * c * FSZ + t0 * FW,
                                    ap=[[FSZ, npart], [1, nf]]),
                        in_=bass.AP(tensor=X[0].tensor,
                                    offset=t0 * FW,
                                    ap=[[FSZ, npart], [1, nf]]))
                    nc.sync.dma_start(
                        out=bass.AP(tensor=X[c].tensor,
                                    offset=(t0 + 1) * FW,
                                    ap=[[FSZ, 16 * c], [1, nf]]),
                        in_=bass.AP(tensor=X[0].tensor,
                                    offset=npart * FSZ + t0 * FW,
                                    ap=[[FSZ, 16 * c], [1, nf]]))

            # Deferred life/flush work: closures drained a few per sweep
            # window so life-broadcast matmuls interleave with dense L1/L2
            # work (no PE-queue stalls on the single pl PSUM slot) and the
            # sigma-copy refresh DMAs overlap compute. Half-1 work of step s
            # drains during the early windows of step s+1.
            pending = []   # items: (min_window_idx, closure)

            def drain(k, wi):
                n = 0
                while pending and n < k and pending[0][0] <= wi:
                    pending.pop(0)[1]()
                    n += 1

            for step in range(steps):
                # --- pre pool ---
                if step == 0:
                    extract_alpha(A_pre, X[0])
                seam_fill_from_A(A_pre)
                pool_half(VMpre, A_pre, 0, HM, HMu, HMd)
                pool_half(VMpre, A_pre, 1, HM, HMu, HMd)

                d_tiles = {}
                d_count = {}
                d_expect = {tb: 8 for tb in range(NBOUT)}
                d_expect[0] = 7
                d_expect[32] = 1

                def life_block(tb, step=step, ntb=2):
                    # one pl tile + one DVE mult covers tbs tb .. tb+ntb-1
                    ntb = min(ntb, NBOUT - tb)
                    plt = ph_pool.tile([128, 2, 256], F32,
                                       name=f"pl_{step}_{tb}", tag="phA")
                    for k in range(ntb):
                        plan = life_plan[tb + k]
                        for i, (half, ridx) in enumerate(plan):
                            nc.tensor.matmul(
                                plt[:, k, :], rt[ridx],
                                LifeF[:, half * 256:half * 256 + 256],
                                start=(i == 0), stop=(i == len(plan) - 1))
                    def xv(t):
                        return bass.AP(tensor=t.tensor,
                                       offset=tb * FW + 1,
                                       ap=[[FSZ, 128], [FW, ntb], [1, 256]])
                    nc.vector.tensor_tensor(xv(X[0]), xv(XN),
                                            plt[:, 0:ntb, :], op=ALU.mult)

                def post_half(half, step=step):
                    extract_alpha(A_post, XN, halves=(half,))
                    pool_half(VMpost, A_post, half, HMp, HMpu, HMpd)
                    qlo = 256 * half
                    qs = slice(qlo, qlo + 256)
                    nc.vector.tensor_tensor(LifeF[:, qs], VMpre[:, qs],
                                            VMpost[:, qs], op=ALU.min)
                    nc.vector.tensor_scalar(LifeF[:, qs], LifeF[:, qs],
                                            0.1, None, op0=ALU.is_gt)
                    if step + 1 < steps:
                        flo = 258 * half + 1
                        nc.vector.tensor_tensor(A_pre[:, flo:flo + 256],
                                                A_post[:, flo:flo + 256],
                                                LifeF[:, qs], op=ALU.mult)
                    if half == 0:
                        # post_half(0) runs during window idx 8; give the
                        # alpha->pool->LifeF chain ~2 windows before the first
                        # life matmul enters the in-order PE queue.
                        for tb in range(0, 12, 2):
                            pending.append((10, lambda tb=tb: life_block(tb, step)))
                        pending.append((10, lambda: (life_block(12, step),
                                                     flush_half(step, 0, 13))))
                        pending.append((10, lambda: life_block(14, step)))
                    else:
                        # drained at the start of the NEXT step; delay past
                        # window 2 so the post pool chain finishes first.
                        for tb in range(16, 22, 2):
                            pending.append((2, lambda tb=tb: life_block(tb, step)))
                        pending.append((2, lambda: flush_half(step, 13, 21)))
                        for tb in range(22, 28, 2):
                            pending.append((2, lambda tb=tb: life_block(tb, step)))
                        pending.append((2, lambda: flush_half(step, 21, 27)))
                        for tb in range(28, NBOUT, 2):
                            pending.append((2, lambda tb=tb: life_block(tb, step)))
                        pending.append((2, lambda: flush_half(step, 27, NB - 1)))

                def l2(rho, ht, hslice):
                    tb, g = rho // 8, rho % 8
                    if tb not in d_tiles:
                        d_tiles[tb] = pd_pool.tile([128, 256], F32,
                                                   name=f"pd_s{step}_{tb}",
                                                   tag="pd")
                        d_count[tb] = 0
                    first = d_count[tb] == 0
                    d_count[tb] += 1
                    last = d_count[tb] == d_expect[tb]
                    nc.tensor.matmul(d_tiles[tb][:], w1t[g][:],
                                     ht[:, hslice], start=first, stop=last)
                    if last:
                        lo = tb * FW + 1
                        nc.vector.tensor_tensor(
                            XN[:, lo:lo + 256], d_tiles[tb][:],
                            X[0][:, lo:lo + 256], op=ALU.add)
                        if tb == 16:
                            seam_fill(XN)
                            post_half(0, step)

                # --- main sweep (L2 deferred one c-iter behind L1, so the
                # L2 matmuls never wait on relu at the head of the PE queue)
                held = []
                for M in range(0, NBOUT + 1, 2):
                    drain(3, M // 2)
                    for c in range(NSIG):
                        rA = 8 * M + 1 - c
                        rB = 8 * M + 5 - c
                        ph_A = ph_pool.tile([128, 2, 256], F32, tag="phA")
                        ph_B = ph_pool.tile([128, 2, 256], F32, tag="phB")
                        for dj in range(3):
                            nc.tensor.matmul(
                                ph_A[:], LW1[0:64, 128 * dj:128 * dj + 128],
                                xwin(X[c], 0, M, dj),
                                start=(dj == 0), stop=(dj == 2))
                            nc.tensor.matmul(
                                ph_B[:], LW1[64:128, 128 * dj:128 * dj + 128],
                                xwin(X[c], 64, M, dj),
                                start=(dj == 0), stop=(dj == 2))
                        ht_A = hpool.tile([128, 512], BF16, tag="htA")
                        relu_tile(ht_A[:], ph_A.rearrange("p a b -> p (a b)"))
                        ht_B = hpool.tile([128, 512], BF16, tag="htB")
                        relu_tile(ht_B[:], ph_B.rearrange("p a b -> p (a b)"))
                        for args in held:
                            l2(*args)
                        held = []
                        for r0, ht in ((rA, ht_A), (rB, ht_B)):
                            if 1 <= r0 <= 256:
                                held.append((r0, ht, slice(0, 256)))
                            if 1 <= r0 + 8 <= 256:
                                held.append((r0 + 8, ht, slice(256, 512)))

                for args in held:
                    l2(*args)
                held = []
                post_half(1, step)
                if step + 1 == steps:
                    drain(len(pending), 10 ** 9)

    nc.compile()
    return nc


_PROGRAM_CACHE = {}


def kernel(x, W0, b0, W1, steps, _trace=False):
    import concourse.bass_utils as bass_utils
    steps = int(steps)
    x = np.asarray(x, dtype=np.float32)
    W0 = np.asarray(W0, dtype=np.float32)
    b0 = np.asarray(b0, dtype=np.float32)
    W1 = np.asarray(W1, dtype=np.float32)
    B = x.shape[0]
    assert x.shape == (8, H, W, C), x.shape

    wts = build_weights(W0, b0, W1)
    key = steps
    if key not in _PROGRAM_CACHE:
        _PROGRAM_CACHE[key] = build_program(steps, wts["life_plan"],
                                            wts["r_stack"].shape[0])
    nc = _PROGRAM_CACHE[key]

    in_maps = []
    for b in range(B):
        xcs = marshal_x(x[b])
        m = {f"xc{c}": xcs[c] for c in range(NSIG)}
        m.update({
            "l1w": wts["l1w"],
            "w1w": wts["w1_stack"],
            "rw": wts["r_stack"],
            "b0w": wts["b0"],
        })
        in_maps.append(m)
    res = bass_utils.run_bass_kernel_spmd(nc, in_maps, list(range(8)),
                                          trace=_trace)
    kernel.last_result = res
    out = np.stack([unmarshal_x(res.results[b]["out"]) for b in range(B)])
    return out.astype(np.float32)
